# revision 10
# baseline (speedup 1.0000x reference)
"""Trainium2 Bass kernel for the LIDAR2D 4-direction selective-scan block.

Sharding: d_inner (E=512) split 8 ways (64 channels/core). The front
(matmuls + BottConv + projections) is cheap and duplicated on every core;
each core then runs the 4-direction selective scan for its 64 channels x
16 states and emits a partial out-projection (B, Dm, L). The host sums the
8 partials and transposes back to (B, L, Dm).

Scan layout per core: tiles of [128 partitions = (nsub in {0,1}) x (64
channels), free = L] — 8 tiles j=0..7 cover states n = 2j + nsub. The
recurrence h_t = exp(A*delta_t)*h_{t-1} + delta_t*B_t*u_t runs in a single
DVE tensor_tensor_scan per tile. y_t = C_t . h_t is a PE matmul with a
0/1 selection matrix contracting the two nsub rows per channel,
accumulated over j in PSUM. Direction permutations (reverse / spatial
transpose) are pure access-pattern tricks on reads/writes.

Perf notes: all front/out-proj matmuls run in fp32r (TF32-like, 1 PE
pass/column vs 4 for fp32); the dbu multiply is offloaded to GPSIMD to
unload DVE, whose tensor_tensor_scan (2 cyc/elem, hardware floor) is the
critical path.
"""

import os
import sys

for _p in ("/opt/trn_rl_repo", os.path.expanduser("~/.axon_site/_ro/trn_rl_repo")):
    if os.path.isdir(_p) and _p not in sys.path:
        sys.path.insert(0, _p)

import numpy as np
import ml_dtypes

import concourse.bass as bass
import concourse.bacc as bacc
import concourse.mybir as mybir
from concourse.tile import TileContext
from concourse.bass_utils import run_bass_kernel_spmd

F32 = mybir.dt.float32
F32R = mybir.dt.float32r
BF16 = mybir.dt.bfloat16
AF = mybir.ActivationFunctionType
OP = mybir.AluOpType

# Problem shape (hardcoded per the harness contract).
B, L, DM, E, N, R, MID, H, W = 2, 2304, 256, 512, 16, 16, 32, 48, 48
NCORES = 8
ESH = E // NCORES          # 64 channels per core
NJ = N // 2                # 8 scan tiles per (b, k); rows = (nsub, e_loc)
HALF = L // 2              # 1152, for PSUM-sized y accumulation

TRACE = bool(os.environ.get("KERNEL_TRACE"))
DEBUG = bool(os.environ.get("KERNEL_DEBUG"))
_LAST_EXEC_NS = None


def _install_profile_shim():
    """Make run_bass_kernel_spmd(trace=True) work in this container:
    register the NTFF hook (antenv.axon_hooks is absent here) and stub
    the S3 artifact upload."""
    import types
    try:
        from antenv.axon_hooks import get_axon_ntff_profile_hook  # noqa: F401
    except ImportError:
        import antenv
        mod = types.ModuleType("antenv.axon_hooks")
        mod._HOOK = None
        mod.set_axon_ntff_profile_hook = lambda h: setattr(mod, "_HOOK", h)
        mod.get_axon_ntff_profile_hook = lambda: mod._HOOK
        sys.modules["antenv.axon_hooks"] = mod
        antenv.axon_hooks = mod
        try:
            from trn_agent_boot.trn_boot import _ntff_profile_via_ctypes
            hook = _ntff_profile_via_ctypes("/opt/axon/libaxon_pjrt.so")
            if hook is not None:
                mod._HOOK = hook
        except Exception as e:  # pragma: no cover
            print(f"profile shim: hook install failed: {e}")
    import concourse.bass_utils as bu
    bu.upload_artifacts = lambda tmpdir: f"file://{tmpdir}"


def _chunks(total, step):
    out = []
    c0 = 0
    while c0 < total:
        out.append((c0, min(step, total - c0)))
        c0 += step
    return out


MM_CHUNKS = _chunks(L, 512)          # matmul free-dim chunks over full L
MM_CHUNKS_HALF = _chunks(HALF, 512)  # chunks within a 1152 half


def build_program():
    nc = bacc.Bacc()

    # ---- DRAM parameters (same shapes on every core; values differ) ----
    xT_d = nc.declare_dram_parameter("xT", [B, DM, L], F32R, isOutput=False)
    posT_d = nc.declare_dram_parameter("posT", [DM, L], F32R, isOutput=False)
    w_in_d = nc.declare_dram_parameter("w_in", [DM, E + ESH], F32R, isOutput=False)
    # pw1 weights padded to [E, 96]: cols 0:32 = w (b=0 -> out rows 0:32),
    # cols 32:64 = 0, cols 64:96 = w (b=1 slice [:, 32:96] -> out rows 0:64
    # with the result at 32:64). fp32r matmul requires dst partition base 0.
    w_pw1_d = nc.declare_dram_parameter("w_pw1", [E, 3 * MID], F32R, isOutput=False)
    pw1b_d = nc.declare_dram_parameter("pw1b", [2 * MID, 1], F32, isOutput=False)
    dwtap_d = nc.declare_dram_parameter("dwtap", [2 * MID, 9], F32, isOutput=False)
    w_pw2_d = nc.declare_dram_parameter("w_pw2", [MID, E], F32R, isOutput=False)
    w_xp_d = nc.declare_dram_parameter("w_xp", [E, 64], F32R, isOutput=False)
    w_dt_d = nc.declare_dram_parameter("w_dt", [R, ESH], F32R, isOutput=False)
    spb_d = nc.declare_dram_parameter("spb", [ESH, 1], F32, isOutput=False)
    ascale_d = nc.declare_dram_parameter("ascale", [2 * ESH, NJ], F32, isOutput=False)
    dire_d = nc.declare_dram_parameter("dire", [ESH, 4], F32, isOutput=False)
    dp4_d = nc.declare_dram_parameter("dp4", [ESH, 1], F32, isOutput=False)
    dpb_d = nc.declare_dram_parameter("dpb", [ESH, 1], F32, isOutput=False)
    w_out_d = nc.declare_dram_parameter("w_out", [ESH, DM], F32R, isOutput=False)
    sel_d = nc.declare_dram_parameter("sel", [2 * ESH, ESH], BF16, isOutput=False)
    out_d = nc.declare_dram_parameter("out", [B, DM, L], F32, isOutput=True)
    dbg = {}
    if DEBUG:
        dbg["xc"] = nc.declare_dram_parameter("dbg_xc", [B, ESH, L], F32,
                                              isOutput=True)
        dbg["delta"] = nc.declare_dram_parameter("dbg_delta", [B, ESH, L], F32,
                                                 isOutput=True)
        dbg["z"] = nc.declare_dram_parameter("dbg_z", [B, ESH, L], BF16,
                                             isOutput=True)
        dbg["y"] = nc.declare_dram_parameter("dbg_y", [B, ESH, L], F32,
                                             isOutput=True)
        dbg["bc"] = nc.declare_dram_parameter("dbg_bc", [2, B, N, L], BF16,
                                              isOutput=True)
        dbg["h1"] = nc.declare_dram_parameter("dbg_h1", [2 * MID, L], F32,
                                              isOutput=True)
        dbg["h2"] = nc.declare_dram_parameter("dbg_h2", [2 * MID, L], F32,
                                              isOutput=True)

    with TileContext(nc) as tc:
        with tc.tile_pool(name="const", bufs=1) as cp:
            # ---- load weights/constants ----
            w_in_t = [cp.tile([128, E + ESH], F32R, tag=f"w_in{t}", name=f"w_in{t}") for t in range(2)]
            for t in range(2):
                nc.sync.dma_start(out=w_in_t[t][:], in_=w_in_d[t * 128:(t + 1) * 128, :])
            w_pw1_t = [cp.tile([128, 3 * MID], F32R, tag=f"w_pw1{t}", name=f"w_pw1{t}") for t in range(4)]
            for t in range(4):
                nc.sync.dma_start(out=w_pw1_t[t][:], in_=w_pw1_d[t * 128:(t + 1) * 128, :])
            pw1b_t = cp.tile([2 * MID, 1], F32, tag="pw1b")
            nc.sync.dma_start(out=pw1b_t[:], in_=pw1b_d[:])
            dwtap_t = cp.tile([2 * MID, 9], F32, tag="dwtap")
            nc.sync.dma_start(out=dwtap_t[:], in_=dwtap_d[:])
            w_pw2_t = cp.tile([MID, E], F32R, tag="w_pw2")
            nc.sync.dma_start(out=w_pw2_t[:], in_=w_pw2_d[:])
            w_xp_t = [cp.tile([128, 64], F32R, tag=f"w_xp{t}", name=f"w_xp{t}") for t in range(4)]
            for t in range(4):
                nc.sync.dma_start(out=w_xp_t[t][:], in_=w_xp_d[t * 128:(t + 1) * 128, :])
            w_dt_t = cp.tile([R, ESH], F32R, tag="w_dt")
            nc.sync.dma_start(out=w_dt_t[:], in_=w_dt_d[:])
            spb_t = cp.tile([ESH, 1], F32, tag="spb")
            nc.sync.dma_start(out=spb_t[:], in_=spb_d[:])
            ascale_t = cp.tile([2 * ESH, NJ], F32, tag="ascale")
            nc.sync.dma_start(out=ascale_t[:], in_=ascale_d[:])
            dire_t = cp.tile([ESH, 4], F32, tag="dire")
            nc.sync.dma_start(out=dire_t[:], in_=dire_d[:])
            dp4_t = cp.tile([ESH, 1], F32, tag="dp4")
            nc.sync.dma_start(out=dp4_t[:], in_=dp4_d[:])
            dpb_t = cp.tile([ESH, 1], F32, tag="dpb")
            nc.sync.dma_start(out=dpb_t[:], in_=dpb_d[:])
            w_out_t = cp.tile([ESH, DM], F32R, tag="w_out")
            nc.sync.dma_start(out=w_out_t[:], in_=w_out_d[:])
            sel_t = cp.tile([2 * ESH, ESH], BF16, tag="sel")
            nc.sync.dma_start(out=sel_t[:], in_=sel_d[:])

            # ---- persistent per-b products of the front ----
            with tc.tile_pool(name="persist", bufs=1) as pp:
                xc_sl = [pp.tile([ESH, L], F32R, tag=f"xc_sl{b}", name=f"xc_sl{b}") for b in range(B)]
                delta_rep = [pp.tile([128, L], F32, tag=f"drep{b}", name=f"drep{b}") for b in range(B)]
                z_sl = [pp.tile([ESH, L], BF16, tag=f"z{b}", name=f"z{b}") for b in range(B)]
                y_acc = [pp.tile([ESH, L], F32, tag=f"yacc{b}", name=f"yacc{b}") for b in range(B)]

                with tc.tile_pool(name="bounce", bufs=1, space="DRAM") as bp:
                    bsrc_t = [bp.tile([N, L], BF16, tag=f"bsrc{b}", name=f"bsrc{b}") for b in range(B)]
                    csrc_t = [bp.tile([N, L], BF16, tag=f"csrc{b}", name=f"csrc{b}") for b in range(B)]

                    _front(nc, tc, locals())
                    _scan_and_out(nc, tc, locals())

    nc.finalize()
    return nc


def _front(nc, tc, env):
    """Positional embed + in-proj + BottConv + projections; fills
    xc_sl / delta_rep / z_sl / bsrc / csrc for both b."""
    g = env
    xT_d, posT_d = g["xT_d"], g["posT_d"]
    w_in_t, w_pw1_t, pw1b_t = g["w_in_t"], g["w_pw1_t"], g["pw1b_t"]
    dwtap_t, w_pw2_t, w_xp_t = g["dwtap_t"], g["w_pw2_t"], g["w_xp_t"]
    w_dt_t, spb_t = g["w_dt_t"], g["spb_t"]
    xc_sl, delta_rep, z_sl = g["xc_sl"], g["delta_rep"], g["z_sl"]
    bsrc_t, csrc_t = g["bsrc_t"], g["csrc_t"]
    # Every core's 64 channels sit at xc rows [0:64] of tile 0: the host
    # permutes pw2_w / W_xproj rows per core so the slice AP is uniform.

    with tc.tile_pool(name="front", bufs=1) as fp, \
         tc.tile_pool(name="fpsum", bufs=2, space="PSUM") as fps:
        h12 = fp.tile([2 * MID, L], F32, tag="h12", name="h12")
        with tc.tile_pool(name="frontio", bufs=1) as iop:
            pos_t = [iop.tile([128, L], F32R, tag=f"pos{t}", name=f"pos{t}")
                     for t in range(2)]
            for t in range(2):
                nc.sync.dma_start(out=pos_t[t][:],
                                  in_=posT_d[t * 128:(t + 1) * 128, :])
            for b in range(B):
                # x^T + pos^T
                xin = [iop.tile([128, L], F32R, tag=f"xin{t}", name=f"xin{t}")
                       for t in range(2)]
                for t in range(2):
                    nc.sync.dma_start(out=xin[t][:],
                                      in_=xT_d[b, t * 128:(t + 1) * 128, :])
                    nc.vector.tensor_tensor(out=xin[t][:], in0=xin[t][:],
                                            in1=pos_t[t][:], op=OP.add)
                # per 512-chunk: xz matmuls -> xh chunk tiles -> pw1 -> h12
                for (c0, nf) in MM_CHUNKS:
                    xhc = [iop.tile([128, 512], F32R, tag=f"xhc{m}", bufs=2,
                                    name=f"xhc{m}") for m in range(4)]
                    for m in range(4):
                        ps = fps.tile([128, 512], F32, tag="fps", bufs=4, name="ps_xz")
                        for kt in range(2):
                            nc.tensor.matmul(ps[:, :nf],
                                             lhsT=w_in_t[kt][:, m * 128:(m + 1) * 128],
                                             rhs=xin[kt][:, c0:c0 + nf],
                                             start=(kt == 0), stop=(kt == 1))
                        nc.scalar.activation(out=xhc[m][:, :nf], in_=ps[:, :nf],
                                             func=AF.Copy)
                    psz = fps.tile([ESH, 512], F32, tag="fps", bufs=4, name="ps_z")
                    for kt in range(2):
                        nc.tensor.matmul(psz[:, :nf],
                                         lhsT=w_in_t[kt][:, E:E + ESH],
                                         rhs=xin[kt][:, c0:c0 + nf],
                                         start=(kt == 0), stop=(kt == 1))
                    nc.scalar.activation(out=z_sl[b][:, c0:c0 + nf],
                                         in_=psz[:, :nf], func=AF.Copy)
                    ps1 = fps.tile([2 * MID, 512], F32, tag="fps", bufs=4, name="ps_pw1")
                    wsl = (slice(0, MID) if b == 0 else slice(MID, 3 * MID))
                    np1 = MID if b == 0 else 2 * MID
                    for kt in range(4):
                        nc.tensor.matmul(ps1[0:np1, :nf],
                                         lhsT=w_pw1_t[kt][:, wsl],
                                         rhs=xhc[kt][:, :nf],
                                         start=(kt == 0), stop=(kt == 3))
                    nc.scalar.activation(
                        out=h12[b * MID:(b + 1) * MID, c0:c0 + nf],
                        in_=ps1[b * MID:(b + 1) * MID, :nf],
                        func=AF.Identity, bias=pw1b_t[b * MID:(b + 1) * MID, :])

        # depthwise 3x3 (both b at once, packed on partitions)
        acc = fp.tile([2 * MID, L], F32R, tag="dwacc", name="dwacc")
        acc3 = acc[:].rearrange("p (h w) -> p h w", w=W)
        h3 = h12[:].rearrange("p (h w) -> p h w", w=W)
        nc.vector.tensor_scalar(out=acc[:], in0=h12[:],
                                scalar1=dwtap_t[:, 4:5], scalar2=None,
                                op0=OP.mult)
        for ky in range(3):
            for kx in range(3):
                if ky == 1 and kx == 1:
                    continue
                dy, dx = ky - 1, kx - 1
                h0, h1 = max(0, -dy), H - max(0, dy)
                w0, w1 = max(0, -dx), W - max(0, dx)
                nc.vector.scalar_tensor_tensor(
                    out=acc3[:, h0:h1, w0:w1],
                    in0=h3[:, h0 + dy:h1 + dy, w0 + dx:w1 + dx],
                    scalar=dwtap_t[:, ky * 3 + kx:ky * 3 + kx + 1],
                    in1=acc3[:, h0:h1, w0:w1],
                    op0=OP.mult, op1=OP.add)

        # matmul requires lhsT/rhs at the same base partition: move b=1's
        # dw rows down to base 0.
        acc_b = [acc[0:MID, :], None]
        acc1 = fp.tile([MID, L], F32R, tag="acc1", name="acc1")
        nc.sync.dma_start(out=acc1[:], in_=acc[MID:2 * MID, :])
        acc_b[1] = acc1[:]
        if g["dbg"]:
            nc.sync.dma_start(out=g["dbg"]["h1"][:], in_=h12[:])
            nc.sync.dma_start(out=g["dbg"]["h2"][:], in_=acc[:].bitcast(F32))
        for b in range(B):
            # pw2 + SiLU -> xc (full E, 4 tiles)
            xc = [fp.tile([128, L], F32R, tag=f"xc{m}", name=f"xc{m}")
                  for m in range(4)]
            for (c0, nf) in MM_CHUNKS:
                for m in range(4):
                    ps2 = fps.tile([128, 512], F32, tag="fps", bufs=4, name="ps_pw2")
                    nc.tensor.matmul(ps2[:, :nf],
                                     lhsT=w_pw2_t[:, m * 128:(m + 1) * 128],
                                     rhs=acc_b[b][:, c0:c0 + nf],
                                     start=True, stop=True)
                    nc.scalar.activation(out=xc[m][:, c0:c0 + nf], in_=ps2[:, :nf],
                                         func=AF.Silu)
            # e-slice of xc for this core (cross-partition move -> DMA)
            nc.sync.dma_start(out=xc_sl[b][:], in_=xc[0][0:ESH, :])
            # x_dbl = xc @ W_xproj  -> [48, L]
            xdbl = fp.tile([64, L], F32R, tag="xdbl", name="xdbl")
            for (c0, nf) in MM_CHUNKS:
                ps3 = fps.tile([64, 512], F32, tag="fps", bufs=4, name="ps_xdbl")
                for kt in range(4):
                    nc.tensor.matmul(ps3[:, :nf],
                                     lhsT=w_xp_t[kt][:],
                                     rhs=xc[kt][:, c0:c0 + nf],
                                     start=(kt == 0), stop=(kt == 3))
                nc.scalar.activation(out=xdbl[:, c0:c0 + nf], in_=ps3[:, :nf],
                                     func=AF.Copy)
            # B/C rows to DRAM (bf16 cast via SWDGE) for later broadcast
            nc.gpsimd.dma_start(out=bsrc_t[b][:],
                                in_=xdbl[R:R + N, :].bitcast(F32))
            nc.gpsimd.dma_start(out=csrc_t[b][:],
                                in_=xdbl[R + N:R + 2 * N, :].bitcast(F32))
            # delta = softplus(dtr @ W_dt + 2*b_dt) into delta_rep rows 0:64
            for (c0, nf) in MM_CHUNKS:
                ps4 = fps.tile([ESH, 512], F32, tag="fps", bufs=4, name="ps_dt")
                nc.tensor.matmul(ps4[:, :nf], lhsT=w_dt_t[:],
                                 rhs=xdbl[0:R, c0:c0 + nf],
                                 start=True, stop=True)
                # softplus(v) = ln(1 + exp(v)); Softplus has no ACT table
                # set, but Exp and Ln share one. |v| stays < ~10 here so
                # exp cannot overflow.
                nc.scalar.activation(out=delta_rep[b][0:ESH, c0:c0 + nf],
                                     in_=ps4[:, :nf],
                                     func=AF.Exp, bias=spb_t[:])
                nc.scalar.activation(out=delta_rep[b][0:ESH, c0:c0 + nf],
                                     in_=delta_rep[b][0:ESH, c0:c0 + nf],
                                     func=AF.Ln, bias=1.0)
            # duplicate rows [0:64] -> [64:128]
            nc.sync.dma_start(out=delta_rep[b][ESH:2 * ESH, :],
                              in_=delta_rep[b][0:ESH, :])
            if g["dbg"]:
                dbg = g["dbg"]
                nc.sync.dma_start(out=dbg["xc"][b], in_=xc_sl[b][:].bitcast(F32))
                nc.sync.dma_start(out=dbg["delta"][b],
                                  in_=delta_rep[b][0:ESH, :])
                nc.sync.dma_start(out=dbg["z"][b], in_=z_sl[b][:])
                nc.sync.dma_start(out=dbg["bc"][0, b], in_=bsrc_t[b][:])
                nc.sync.dma_start(out=dbg["bc"][1, b], in_=csrc_t[b][:])


def _scan_and_out(nc, tc, env):
    g = env
    xc_sl, delta_rep, z_sl, y_acc = g["xc_sl"], g["delta_rep"], g["z_sl"], g["y_acc"]
    bsrc_t, csrc_t = g["bsrc_t"], g["csrc_t"]
    ascale_t, dire_t = g["ascale_t"], g["dire_t"]
    dp4_t, dpb_t = g["dp4_t"], g["dpb_t"]
    sel_t, w_out_t, out_d = g["sel_t"], g["w_out_t"], g["out_d"]

    with tc.tile_pool(name="scan", bufs=1) as sp, \
         tc.tile_pool(name="work", bufs=3) as wp, \
         tc.tile_pool(name="ypsum", bufs=1, space="PSUM") as yps, \
         tc.tile_pool(name="opsum", bufs=2, space="PSUM") as ops:
        for b in range(B):
            # init y_acc with the D*u skip term: Dp*(4*xc + sum_k dir_k)
            nc.scalar.activation(out=y_acc[b][:], in_=xc_sl[b][:],
                                 func=AF.Identity,
                                 bias=dpb_t[:], scale=dp4_t[:])
            dA_t = [None] * NJ
            for k in range(4):
                # u_k = perm_k(xc) + dir_k ; du = delta * u_k (bf16)
                xc3 = xc_sl[b][:].rearrange("p (h w) -> p h w", w=W)
                if k == 0:
                    src = xc3
                elif k == 1:
                    src = xc3[:, ::-1, ::-1]
                elif k == 2:
                    src = xc_sl[b][:].rearrange("p (h w) -> p w h", w=W)
                else:
                    src = xc_sl[b][:].rearrange("p (h w) -> p w h", w=W)[:, ::-1, ::-1]
                u_tmp = wp.tile([ESH, L], BF16, tag="u_tmp", bufs=2, name="u_tmp")
                u3 = u_tmp[:].rearrange("p (a c) -> p a c", c=W)
                nc.scalar.activation(out=u3, in_=src, func=AF.Identity,
                                     bias=dire_t[:, k:k + 1])
                du_rep = wp.tile([128, L], BF16, tag="du_rep", bufs=2, name="du_rep")
                nc.vector.tensor_tensor(out=du_rep[0:ESH, :],
                                        in0=delta_rep[b][0:ESH, :],
                                        in1=u_tmp[:], op=OP.mult)
                nc.sync.dma_start(out=du_rep[ESH:2 * ESH, :],
                                  in_=du_rep[0:ESH, :])
                ypsum = [yps.tile([ESH, HALF], F32, tag=f"yps{h}", name=f"yps{h}")
                         for h in range(2)]
                for j in range(NJ):
                    if k == 0:
                        dA_t[j] = sp.tile([128, L], F32 if j == 0 else BF16,
                                          tag=f"dA{j}", name=f"dA{j}")
                        nc.scalar.activation(out=dA_t[j][:], in_=delta_rep[b][:],
                                             func=AF.Exp,
                                             scale=ascale_t[:, j:j + 1])
                    B_t = wp.tile([128, L], BF16, tag="B_t", bufs=2, name="B_t")
                    C_t = wp.tile([128, L], BF16, tag="C_t", bufs=2, name="C_t")
                    for ns in range(2):
                        nc.sync.dma_start(
                            out=B_t[ns * ESH:(ns + 1) * ESH, :],
                            in_=bsrc_t[b][2 * j + ns:2 * j + ns + 1, :]
                            .to_broadcast((ESH, L)))
                        nc.sync.dma_start(
                            out=C_t[ns * ESH:(ns + 1) * ESH, :],
                            in_=csrc_t[b][2 * j + ns:2 * j + ns + 1, :]
                            .to_broadcast((ESH, L)))
                    dbu = wp.tile([128, L], BF16, tag="workA", bufs=2, name="dbu")
                    nc.gpsimd.tensor_tensor(out=dbu[:], in0=du_rep[:],
                                            in1=B_t[:], op=OP.mult)
                    h_t = wp.tile([128, L], BF16, tag="workH", bufs=2, name="h_t")
                    nc.vector.tensor_tensor_scan(out=h_t[:], data0=dA_t[j][:],
                                                 data1=dbu[:], initial=0.0,
                                                 op0=OP.mult, op1=OP.add)
                    hc = wp.tile([128, L], BF16, tag="workC", bufs=2, name="hc")
                    nc.vector.tensor_tensor(out=hc[:], in0=h_t[:],
                                            in1=C_t[:], op=OP.mult)
                    for hh in range(2):
                        for (c0, nf) in MM_CHUNKS_HALF:
                            nc.tensor.matmul(
                                ypsum[hh][:, c0:c0 + nf],
                                lhsT=sel_t[:],
                                rhs=hc[:, hh * HALF + c0:hh * HALF + c0 + nf],
                                start=(j == 0), stop=(j == NJ - 1))
                # accumulate un-permuted ys_k into y_acc
                for hh in range(2):
                    pv = ypsum[hh][:]
                    if k == 0:
                        dst = y_acc[b][:, hh * HALF:(hh + 1) * HALF]
                        srcv = pv
                    elif k == 1:
                        dst = y_acc[b][:, (1 - hh) * HALF:(2 - hh) * HALF]
                        srcv = pv[:, ::-1]
                    elif k == 2:
                        # ys[i], i=a*48+b_ -> l = b_*48+a ; half hh: a in [24hh,24hh+24)
                        dst = y_acc[b][:].rearrange("p (bb a) -> p bb a", a=W)[
                            :, :, 24 * hh:24 * hh + 24]
                        srcv = pv.rearrange("p (a bb) -> p bb a", bb=W)
                    else:
                        dst = y_acc[b][:].rearrange("p (bb a) -> p bb a", a=W)[
                            :, :, 24 * (1 - hh):24 * (1 - hh) + 24]
                        srcv = pv.rearrange("p (a bb) -> p bb a", bb=W)[:, ::-1, ::-1]
                    nc.vector.tensor_tensor(out=dst, in0=srcv, in1=dst, op=OP.add)
            if g["dbg"]:
                nc.sync.dma_start(out=g["dbg"]["y"][b], in_=y_acc[b][:])
            # y_fin = y_acc * silu(z); out_partial = W_out^T @ y_fin
            sz = wp.tile([ESH, L], BF16, tag="u_tmp", bufs=2, name="sz")
            nc.scalar.activation(out=sz[:], in_=z_sl[b][:], func=AF.Silu)
            yv = wp.tile([ESH, L], F32R, tag="yv", bufs=1, name="yv")
            nc.vector.tensor_tensor(out=yv[:], in0=y_acc[b][:], in1=sz[:],
                                    op=OP.mult)
            for m in range(2):
                osb = wp.tile([128, L], F32, tag="osb", bufs=2, name="osb")
                for (c0, nf) in MM_CHUNKS:
                    po = ops.tile([128, 512], F32, tag="out", name="ps_out")
                    nc.tensor.matmul(po[:, :nf],
                                     lhsT=w_out_t[:, m * 128:(m + 1) * 128],
                                     rhs=yv[:, c0:c0 + nf],
                                     start=True, stop=True)
                    nc.scalar.activation(out=osb[:, c0:c0 + nf], in_=po[:, :nf],
                                         func=AF.Copy)
                nc.sync.dma_start(out=out_d[b, m * 128:(m + 1) * 128, :],
                                  in_=osb[:])


def _r32r(a):
    """Round fp32 -> fp32r (TF32-like, 10 explicit mantissa bits)."""
    b = np.ascontiguousarray(a, np.float32).view(np.uint32)
    return (((b.astype(np.uint64) + 0x1000) & 0xFFFFE000)
            .astype(np.uint32).view(np.float32))


def _host_prep(inputs):
    x = np.asarray(inputs["x"], np.float32)
    W_pos = np.asarray(inputs["W_pos"], np.float32)
    b_pos = np.asarray(inputs["b_pos"], np.float32)
    W_in = np.asarray(inputs["W_in"], np.float32)
    pw1_w = np.asarray(inputs["pw1_w"], np.float32)
    pw1_b = np.asarray(inputs["pw1_b"], np.float32)
    dw_w = np.asarray(inputs["dw_w"], np.float32)
    pw2_w = np.asarray(inputs["pw2_w"], np.float32)
    W_xproj = np.asarray(inputs["W_xproj"], np.float32)
    W_dt = np.asarray(inputs["W_dt"], np.float32)
    b_dt = np.asarray(inputs["b_dt"], np.float32)
    A_log = np.asarray(inputs["A_log"], np.float32)
    Dp = np.asarray(inputs["Dp"], np.float32)
    dir_emb = np.asarray(inputs["dir_emb"], np.float32)
    W_out = np.asarray(inputs["W_out"], np.float32)

    gy, gx = np.meshgrid(np.arange(H, dtype=np.float32),
                         np.arange(W, dtype=np.float32), indexing="ij")
    coords = np.stack([gy, gx], -1) / (H - 1) * 2 - 1
    pos = (coords.reshape(L, 2) @ W_pos + b_pos).astype(np.float32)

    common = {
        "xT": _r32r(x.transpose(0, 2, 1)),
        "posT": _r32r(pos.T),
        "w_pw1": _r32r(np.concatenate(
            [pw1_w.reshape(MID, E).T, np.zeros((E, MID), np.float32),
             pw1_w.reshape(MID, E).T], axis=1)),
        "pw1b": np.ascontiguousarray(
            np.concatenate([pw1_b, pw1_b]).reshape(2 * MID, 1)),
        "dwtap": np.ascontiguousarray(
            np.concatenate([dw_w.reshape(MID, 9)] * 2, axis=0)),
    }
    w_pw2_base = pw2_w.reshape(E, MID).T  # (MID, E)
    A = -np.exp(A_log)  # (E, N)

    sel = np.zeros((2 * ESH, ESH), np.float32)
    for p in range(2 * ESH):
        sel[p, p % ESH] = 1.0
    sel = sel.astype(ml_dtypes.bfloat16)

    in_maps = []
    for c in range(NCORES):
        e0 = c * ESH
        sl = slice(e0, e0 + ESH)
        A_sl = A[sl]  # (64, 16)
        ascale = np.empty((2 * ESH, NJ), np.float32)
        for p in range(2 * ESH):
            for j in range(NJ):
                ascale[p, j] = A_sl[p % ESH, 2 * j + p // ESH]
        m = dict(common)
        # channel permutation putting this core's slice at rows [0:64]
        perm = np.concatenate([np.arange(e0, e0 + ESH),
                               np.arange(0, e0),
                               np.arange(e0 + ESH, E)])
        m["w_pw2"] = _r32r(w_pw2_base[:, perm])
        m["w_xp"] = _r32r(np.concatenate(
            [W_xproj[perm, :], np.zeros((E, 64 - (R + 2 * N)), np.float32)],
            axis=1))
        m["w_in"] = _r32r(
            np.concatenate([W_in[:, :E], W_in[:, E + e0:E + e0 + ESH]], axis=1))
        m["w_dt"] = _r32r(W_dt[:, sl])
        m["spb"] = np.ascontiguousarray((2.0 * b_dt[sl]).reshape(ESH, 1))
        m["ascale"] = ascale
        m["dire"] = np.ascontiguousarray(dir_emb[:, sl].T)
        m["dp4"] = np.ascontiguousarray((4.0 * Dp[sl]).reshape(ESH, 1))
        m["dpb"] = np.ascontiguousarray(
            (Dp[sl] * dir_emb[:, sl].sum(0)).reshape(ESH, 1))
        m["w_out"] = _r32r(W_out[sl, :])
        m["sel"] = sel
        in_maps.append(m)
    return in_maps


_PROGRAM = None
_LAST_RESULTS = None
_LAST_INSTS = None


def _get_program():
    global _PROGRAM
    if _PROGRAM is None:
        _PROGRAM = build_program()
    return _PROGRAM


def kernel(**inputs):
    global _LAST_EXEC_NS, _LAST_RESULTS
    assert int(inputs["H"]) == H and int(inputs["W"]) == W
    in_maps = _host_prep(inputs)
    if TRACE:
        _install_profile_shim()
    res = run_bass_kernel_spmd(_get_program(), in_maps,
                               list(range(NCORES)), trace=TRACE)
    _LAST_EXEC_NS = res.exec_time_ns
    _LAST_RESULTS = res.results
    global _LAST_INSTS
    _LAST_INSTS = res.instructions_and_trace
    out = np.zeros((B, DM, L), np.float32)
    for r in res.results:
        out += np.asarray(r["out"], np.float32)
    return np.ascontiguousarray(out.transpose(0, 2, 1))


# revision 13
# speedup vs baseline: 1.2190x; 1.2190x over previous
"""Trainium2 Bass kernel for the LIDAR2D 4-direction selective-scan block.

Sharding: d_inner (E=512) split 8 ways (64 channels/core). The front
(matmuls + BottConv + projections) is cheap and duplicated on every core;
each core then runs the 4-direction selective scan for its 64 channels x
16 states and emits a partial out-projection (B, Dm, L). The host sums the
8 partials and transposes back to (B, L, Dm).

Scan layout per core: tiles of [128 partitions = (nsub in {0,1}) x (64
channels), free = L] — 8 tiles j=0..7 cover states n = 2j + nsub. The
recurrence h_t = exp(A*delta_t)*h_{t-1} + delta_t*B_t*u_t runs in a single
DVE tensor_tensor_scan per tile (2 cyc/elem — the hardware floor and the
kernel's critical path). y_t = C_t . h_t is a PE matmul with a 0/1
selection matrix contracting the two nsub rows per channel, accumulated
over j in PSUM. Direction permutations (reverse / spatial transpose) are
pure access-pattern tricks on reads/writes.

Perf structure: the front runs in bf16 (1 PE pass/col) with per-512-chunk
transient tiles so its SBUF footprint is small enough to coexist with the
scan working set. Emission order overlaps front(b=1) compute under
scan(b=0)'s DVE stream: front(0); stageA(1); scan(0,k=0); conv+stageB(1);
scan(0,k=1..3); scan(1). The out-projection runs in fp32r.
"""

import os
import sys

for _p in ("/opt/trn_rl_repo", os.path.expanduser("~/.axon_site/_ro/trn_rl_repo")):
    if os.path.isdir(_p) and _p not in sys.path:
        sys.path.insert(0, _p)

import numpy as np
import ml_dtypes

import concourse.bass as bass
import concourse.bacc as bacc
import concourse.mybir as mybir
from concourse.tile import TileContext
from concourse.bass_utils import run_bass_kernel_spmd

F32 = mybir.dt.float32
F32R = mybir.dt.float32r
BF16 = mybir.dt.bfloat16
AF = mybir.ActivationFunctionType
OP = mybir.AluOpType

# Problem shape (hardcoded per the harness contract).
B, L, DM, E, N, R, MID, H, W = 2, 2304, 256, 512, 16, 16, 32, 48, 48
NCORES = 8
ESH = E // NCORES          # 64 channels per core
NJ = N // 2                # 8 scan tiles per (b, k); rows = (nsub, e_loc)
HALF = L // 2              # 1152, for PSUM-sized y accumulation

TRACE = bool(os.environ.get("KERNEL_TRACE"))
_LAST_EXEC_NS = None


def _install_profile_shim():
    """Make run_bass_kernel_spmd(trace=True) work in this container:
    register the NTFF hook (antenv.axon_hooks is absent here) and stub
    the S3 artifact upload."""
    import types
    try:
        from antenv.axon_hooks import get_axon_ntff_profile_hook  # noqa: F401
    except ImportError:
        import antenv
        mod = types.ModuleType("antenv.axon_hooks")
        mod._HOOK = None
        mod.set_axon_ntff_profile_hook = lambda h: setattr(mod, "_HOOK", h)
        mod.get_axon_ntff_profile_hook = lambda: mod._HOOK
        sys.modules["antenv.axon_hooks"] = mod
        antenv.axon_hooks = mod
        try:
            from trn_agent_boot.trn_boot import _ntff_profile_via_ctypes
            hook = _ntff_profile_via_ctypes("/opt/axon/libaxon_pjrt.so")
            if hook is not None:
                mod._HOOK = hook
        except Exception as e:  # pragma: no cover
            print(f"profile shim: hook install failed: {e}")
    import concourse.bass_utils as bu
    bu.upload_artifacts = lambda tmpdir: f"file://{tmpdir}"


def _chunks(total, step):
    out = []
    c0 = 0
    while c0 < total:
        out.append((c0, min(step, total - c0)))
        c0 += step
    return out


MM_CHUNKS = _chunks(L, 512)          # matmul free-dim chunks over full L
MM_CHUNKS_HALF = _chunks(HALF, 512)  # chunks within a 1152 half


class Env:
    pass


def build_program():
    nc = bacc.Bacc()
    g = Env()
    g.nc = nc

    # ---- DRAM parameters (same shapes on every core; values differ) ----
    g.xT_d = nc.declare_dram_parameter("xT", [B, DM, L], BF16, isOutput=False)
    g.posT_d = nc.declare_dram_parameter("posT", [DM, L], BF16, isOutput=False)
    g.w_in_d = nc.declare_dram_parameter("w_in", [DM, E + ESH], BF16, isOutput=False)
    g.w_pw1_d = nc.declare_dram_parameter("w_pw1", [E, MID], BF16, isOutput=False)
    g.pw1b_d = nc.declare_dram_parameter("pw1b", [MID, 1], F32, isOutput=False)
    g.dwtap_d = nc.declare_dram_parameter("dwtap", [MID, 9], F32, isOutput=False)
    g.w_pw2_d = nc.declare_dram_parameter("w_pw2", [MID, E], BF16, isOutput=False)
    g.w_xp_d = nc.declare_dram_parameter("w_xp", [E, 64], BF16, isOutput=False)
    g.w_dt_d = nc.declare_dram_parameter("w_dt", [R, ESH], BF16, isOutput=False)
    g.spb_d = nc.declare_dram_parameter("spb", [ESH, 1], F32, isOutput=False)
    g.ascale_d = nc.declare_dram_parameter("ascale", [2 * ESH, NJ], F32, isOutput=False)
    g.dire_d = nc.declare_dram_parameter("dire", [ESH, 4], F32, isOutput=False)
    g.dp4_d = nc.declare_dram_parameter("dp4", [ESH, 1], F32, isOutput=False)
    g.dpb_d = nc.declare_dram_parameter("dpb", [ESH, 1], F32, isOutput=False)
    g.w_out_d = nc.declare_dram_parameter("w_out", [ESH, DM], BF16, isOutput=False)
    g.sel_d = nc.declare_dram_parameter("sel", [2 * ESH, ESH], BF16, isOutput=False)
    g.out_d = nc.declare_dram_parameter("out", [B, DM, L], F32, isOutput=True)

    with TileContext(nc) as tc:
        g.tc = tc
        with tc.tile_pool(name="const", bufs=1) as cp, \
             tc.tile_pool(name="persist", bufs=1) as pp, \
             tc.tile_pool(name="front", bufs=1) as fp, \
             tc.tile_pool(name="scan", bufs=1) as sp, \
             tc.tile_pool(name="work", bufs=2) as wp, \
             tc.tile_pool(name="fpsum", bufs=2, space="PSUM") as fps, \
             tc.tile_pool(name="ypsum", bufs=1, space="PSUM") as yps, \
             tc.tile_pool(name="bounce", bufs=1, space="DRAM") as bp:
            g.cp, g.pp, g.fp, g.sp, g.wp, g.fps, g.yps, g.bp = \
                cp, pp, fp, sp, wp, fps, yps, bp
            _load_consts(g)

            # persistent per-b products of the front
            g.xc_sl = [pp.tile([ESH, L], BF16, tag=f"xc_sl{b}", name=f"xc_sl{b}") for b in range(B)]
            g.delta_rep = [pp.tile([128, L], F32, tag=f"drep{b}", name=f"drep{b}") for b in range(B)]
            g.z_sl = [pp.tile([ESH, L], BF16, tag=f"z{b}", name=f"z{b}") for b in range(B)]
            g.y_acc = [pp.tile([ESH, L], F32, tag=f"yacc{b}", name=f"yacc{b}") for b in range(B)]
            g.bsrc = [bp.tile([N, L], BF16, tag=f"bsrc{b}", name=f"bsrc{b}") for b in range(B)]
            g.csrc = [bp.tile([N, L], BF16, tag=f"csrc{b}", name=f"csrc{b}") for b in range(B)]
            # h12/acc full-L per-b (conv needs the whole plane)
            g.h12 = fp.tile([MID, L], BF16, tag="h12", name="h12")
            g.accB = fp.tile([MID, L], BF16, tag="accB", name="accB")
            g.xdbl = fp.tile([64, L], BF16, tag="xdbl", name="xdbl")
            g.dA = [None] * NJ
            g.du = [None] * 4

            # ---- emission schedule (overlap front(1) under scan(0)) ----
            _front_stageA(g, 0)
            _front_conv(g, 0)
            _front_stageB(g, 0)
            _front_stageA(g, 1)
            _scan_prep(g, 0)
            _scan_k(g, 0, 0)
            _front_conv(g, 1)
            _front_stageB(g, 1)
            for k in range(1, 4):
                _scan_k(g, 0, k)
            _finish_b(g, 0)
            _scan_prep(g, 1)
            for k in range(4):
                _scan_k(g, 1, k)
            _finish_b(g, 1)

    nc.finalize()
    return nc


def _load_consts(g):
    nc, cp = g.nc, g.cp
    g.w_in_t = [cp.tile([128, E + ESH], BF16, tag=f"w_in{t}", name=f"w_in{t}") for t in range(2)]
    for t in range(2):
        nc.sync.dma_start(out=g.w_in_t[t][:], in_=g.w_in_d[t * 128:(t + 1) * 128, :])
    g.w_pw1_t = [cp.tile([128, MID], BF16, tag=f"w_pw1{t}", name=f"w_pw1{t}") for t in range(4)]
    for t in range(4):
        nc.sync.dma_start(out=g.w_pw1_t[t][:], in_=g.w_pw1_d[t * 128:(t + 1) * 128, :])
    g.pw1b_t = cp.tile([MID, 1], F32, tag="pw1b", name="pw1b")
    nc.sync.dma_start(out=g.pw1b_t[:], in_=g.pw1b_d[:])
    g.dwtap_t = cp.tile([MID, 9], F32, tag="dwtap", name="dwtap")
    nc.sync.dma_start(out=g.dwtap_t[:], in_=g.dwtap_d[:])
    g.w_pw2_t = cp.tile([MID, E], BF16, tag="w_pw2", name="w_pw2")
    nc.sync.dma_start(out=g.w_pw2_t[:], in_=g.w_pw2_d[:])
    g.w_xp_t = [cp.tile([128, 64], BF16, tag=f"w_xp{t}", name=f"w_xp{t}") for t in range(4)]
    for t in range(4):
        nc.sync.dma_start(out=g.w_xp_t[t][:], in_=g.w_xp_d[t * 128:(t + 1) * 128, :])
    g.w_dt_t = cp.tile([R, ESH], BF16, tag="w_dt", name="w_dt")
    nc.sync.dma_start(out=g.w_dt_t[:], in_=g.w_dt_d[:])
    g.spb_t = cp.tile([ESH, 1], F32, tag="spb", name="spb")
    nc.sync.dma_start(out=g.spb_t[:], in_=g.spb_d[:])
    g.ascale_t = cp.tile([2 * ESH, NJ], F32, tag="ascale", name="ascale")
    nc.sync.dma_start(out=g.ascale_t[:], in_=g.ascale_d[:])
    g.dire_t = cp.tile([ESH, 4], F32, tag="dire", name="dire")
    nc.sync.dma_start(out=g.dire_t[:], in_=g.dire_d[:])
    g.dp4_t = cp.tile([ESH, 1], F32, tag="dp4", name="dp4")
    nc.sync.dma_start(out=g.dp4_t[:], in_=g.dp4_d[:])
    g.dpb_t = cp.tile([ESH, 1], F32, tag="dpb", name="dpb")
    nc.sync.dma_start(out=g.dpb_t[:], in_=g.dpb_d[:])
    g.w_out_t = cp.tile([ESH, DM], BF16, tag="w_out", name="w_out")
    nc.sync.dma_start(out=g.w_out_t[:], in_=g.w_out_d[:])
    g.sel_t = cp.tile([2 * ESH, ESH], BF16, tag="sel", name="sel")
    nc.sync.dma_start(out=g.sel_t[:], in_=g.sel_d[:])


def _front_stageA(g, b):
    """x+pos -> xz -> (z slice, xh) -> pw1 -> h12 (full [MID, L])."""
    nc, fp, fps = g.nc, g.fp, g.fps
    for (c0, nf) in MM_CHUNKS:
        xin = [fp.tile([128, 512], BF16, tag=f"xin{t}", bufs=2, name=f"xin{t}") for t in range(2)]
        for t in range(2):
            posc = fp.tile([128, 512], BF16, tag=f"posc{t}", bufs=1)
            nc.sync.dma_start(out=posc[:, :nf],
                              in_=g.posT_d[t * 128:(t + 1) * 128, c0:c0 + nf])
            nc.sync.dma_start(out=xin[t][:, :nf],
                              in_=g.xT_d[b, t * 128:(t + 1) * 128, c0:c0 + nf])
            nc.vector.tensor_tensor(out=xin[t][:, :nf], in0=xin[t][:, :nf],
                                    in1=posc[:, :nf], op=OP.add)
        xhc = [fp.tile([128, 512], BF16, tag=f"xhc{m}", bufs=2, name=f"xhc{m}") for m in range(4)]
        for m in range(4):
            ps = fps.tile([128, 512], F32, tag="fps", bufs=2, name="ps_xz")
            for kt in range(2):
                nc.tensor.matmul(ps[:, :nf],
                                 lhsT=g.w_in_t[kt][:, m * 128:(m + 1) * 128],
                                 rhs=xin[kt][:, :nf],
                                 start=(kt == 0), stop=(kt == 1))
            nc.scalar.activation(out=xhc[m][:, :nf], in_=ps[:, :nf], func=AF.Copy)
        psz = fps.tile([ESH, 512], F32, tag="fps", bufs=2, name="ps_z")
        for kt in range(2):
            nc.tensor.matmul(psz[:, :nf],
                             lhsT=g.w_in_t[kt][:, E:E + ESH],
                             rhs=xin[kt][:, :nf],
                             start=(kt == 0), stop=(kt == 1))
        nc.scalar.activation(out=g.z_sl[b][:, c0:c0 + nf], in_=psz[:, :nf],
                             func=AF.Copy)
        ps1 = fps.tile([MID, 512], F32, tag="fps", bufs=2, name="ps_pw1")
        for kt in range(4):
            nc.tensor.matmul(ps1[:, :nf], lhsT=g.w_pw1_t[kt][:],
                             rhs=xhc[kt][:, :nf],
                             start=(kt == 0), stop=(kt == 3))
        nc.scalar.activation(out=g.h12[:, c0:c0 + nf], in_=ps1[:, :nf],
                             func=AF.Identity, bias=g.pw1b_t[:])


def _front_conv(g, b):
    """Depthwise 3x3 on h12 -> accB (bf16 copy for pw2 rhs)."""
    nc, fp = g.nc, g.fp
    acc = fp.tile([MID, L], F32, tag="dwacc")
    acc3 = acc[:].rearrange("p (h w) -> p h w", w=W)
    h3 = g.h12[:].rearrange("p (h w) -> p h w", w=W)
    nc.vector.tensor_scalar(out=acc[:], in0=g.h12[:],
                            scalar1=g.dwtap_t[:, 4:5], scalar2=None,
                            op0=OP.mult)
    for ky in range(3):
        for kx in range(3):
            if ky == 1 and kx == 1:
                continue
            dy, dx = ky - 1, kx - 1
            h0, h1 = max(0, -dy), H - max(0, dy)
            w0, w1 = max(0, -dx), W - max(0, dx)
            nc.vector.scalar_tensor_tensor(
                out=acc3[:, h0:h1, w0:w1],
                in0=h3[:, h0 + dy:h1 + dy, w0 + dx:w1 + dx],
                scalar=g.dwtap_t[:, ky * 3 + kx:ky * 3 + kx + 1],
                in1=acc3[:, h0:h1, w0:w1],
                op0=OP.mult, op1=OP.add)
    nc.scalar.activation(out=g.accB[:], in_=acc[:], func=AF.Copy)


def _front_stageB(g, b):
    """pw2+SiLU -> xc chunks -> (xc_sl slice, x_dbl, delta, B/C to DRAM)."""
    nc, fp, fps = g.nc, g.fp, g.fps
    for (c0, nf) in MM_CHUNKS:
        xcc = [fp.tile([128, 512], BF16, tag=f"xcc{m}", bufs=2, name=f"xcc{m}") for m in range(4)]
        for m in range(4):
            ps2 = fps.tile([128, 512], F32, tag="fps", bufs=2, name="ps_pw2")
            nc.tensor.matmul(ps2[:, :nf],
                             lhsT=g.w_pw2_t[:, m * 128:(m + 1) * 128],
                             rhs=g.accB[:, c0:c0 + nf], start=True, stop=True)
            nc.scalar.activation(out=xcc[m][:, :nf], in_=ps2[:, :nf],
                                 func=AF.Silu)
        nc.sync.dma_start(out=g.xc_sl[b][:, c0:c0 + nf], in_=xcc[0][0:ESH, :nf])
        ps3 = fps.tile([64, 512], F32, tag="fps", bufs=2, name="ps_xdbl")
        for kt in range(4):
            nc.tensor.matmul(ps3[:, :nf], lhsT=g.w_xp_t[kt][:],
                             rhs=xcc[kt][:, :nf],
                             start=(kt == 0), stop=(kt == 3))
        nc.scalar.activation(out=g.xdbl[:, c0:c0 + nf], in_=ps3[:, :nf],
                             func=AF.Copy)
        ps4 = fps.tile([ESH, 512], F32, tag="fps", bufs=2, name="ps_dt")
        nc.tensor.matmul(ps4[:, :nf], lhsT=g.w_dt_t[:],
                         rhs=g.xdbl[0:R, c0:c0 + nf], start=True, stop=True)
        # softplus(v) = ln(1 + exp(v)); |v| < ~10 here so exp cannot overflow
        nc.scalar.activation(out=g.delta_rep[b][0:ESH, c0:c0 + nf],
                             in_=ps4[:, :nf], func=AF.Exp, bias=g.spb_t[:])
        nc.scalar.activation(out=g.delta_rep[b][0:ESH, c0:c0 + nf],
                             in_=g.delta_rep[b][0:ESH, c0:c0 + nf],
                             func=AF.Ln, bias=1.0)
    # B/C rows to DRAM for later partition-broadcast loads
    nc.sync.dma_start(out=g.bsrc[b][:], in_=g.xdbl[R:R + N, :])
    nc.sync.dma_start(out=g.csrc[b][:], in_=g.xdbl[R + N:R + 2 * N, :])
    # duplicate delta rows [0:64] -> [64:128]
    nc.sync.dma_start(out=g.delta_rep[b][ESH:2 * ESH, :],
                      in_=g.delta_rep[b][0:ESH, :])


def _scan_prep(g, b):
    """y_acc init (D*u skip), dA exps, per-direction du tiles."""
    nc, sp, wp = g.nc, g.sp, g.wp
    nc.scalar.activation(out=g.y_acc[b][:], in_=g.xc_sl[b][:],
                         func=AF.Identity, bias=g.dpb_t[:], scale=g.dp4_t[:])
    for j in range(NJ):
        g.dA[j] = sp.tile([128, L], BF16, tag=f"dA{j}", name=f"dA{j}")
        nc.scalar.activation(out=g.dA[j][:], in_=g.delta_rep[b][:],
                             func=AF.Exp, scale=g.ascale_t[:, j:j + 1])
    xc3 = g.xc_sl[b][:].rearrange("p (h w) -> p h w", w=W)
    xcT = g.xc_sl[b][:].rearrange("p (h w) -> p w h", w=W)
    for k in range(4):
        src = [xc3, xc3[:, ::-1, ::-1], xcT, xcT[:, ::-1, ::-1]][k]
        u_tmp = wp.tile([ESH, L], BF16, tag="u_tmp", bufs=1)
        u3 = u_tmp[:].rearrange("p (a c) -> p a c", c=W)
        nc.scalar.activation(out=u3, in_=src, func=AF.Identity,
                             bias=g.dire_t[:, k:k + 1])
        g.du[k] = sp.tile([128, L], BF16, tag=f"du{k}", name=f"du{k}")
        nc.vector.tensor_tensor(out=g.du[k][0:ESH, :],
                                in0=g.delta_rep[b][0:ESH, :],
                                in1=u_tmp[:], op=OP.mult)
        nc.sync.dma_start(out=g.du[k][ESH:2 * ESH, :], in_=g.du[k][0:ESH, :])


def _scan_k(g, b, k):
    """One direction's 8 scan tiles + y accumulation."""
    nc, wp, yps = g.nc, g.wp, g.yps
    ypsum = [yps.tile([ESH, HALF], F32, tag=f"yps{h}", name=f"yps{h}") for h in range(2)]
    for j in range(NJ):
        B_t = wp.tile([128, L], BF16, tag="B_t", bufs=2)
        C_t = wp.tile([128, L], BF16, tag="C_t", bufs=2)
        for ns in range(2):
            nc.sync.dma_start(
                out=B_t[ns * ESH:(ns + 1) * ESH, :],
                in_=g.bsrc[b][2 * j + ns:2 * j + ns + 1, :]
                .to_broadcast((ESH, L)))
            nc.sync.dma_start(
                out=C_t[ns * ESH:(ns + 1) * ESH, :],
                in_=g.csrc[b][2 * j + ns:2 * j + ns + 1, :]
                .to_broadcast((ESH, L)))
        dbu = wp.tile([128, L], BF16, tag="workA", bufs=2)
        nc.vector.tensor_tensor(out=dbu[:], in0=g.du[k][:], in1=B_t[:],
                                op=OP.mult)
        h_t = wp.tile([128, L], BF16, tag="workH", bufs=2)
        nc.vector.tensor_tensor_scan(out=h_t[:], data0=g.dA[j][:],
                                     data1=dbu[:], initial=0.0,
                                     op0=OP.mult, op1=OP.add)
        hc = wp.tile([128, L], BF16, tag="workA", bufs=2)
        nc.vector.tensor_tensor(out=hc[:], in0=h_t[:], in1=C_t[:], op=OP.mult)
        for hh in range(2):
            for (c0, nf) in MM_CHUNKS_HALF:
                nc.tensor.matmul(
                    ypsum[hh][:, c0:c0 + nf],
                    lhsT=g.sel_t[:],
                    rhs=hc[:, hh * HALF + c0:hh * HALF + c0 + nf],
                    start=(j == 0), stop=(j == NJ - 1))
    # accumulate un-permuted ys_k into y_acc
    for hh in range(2):
        pv = ypsum[hh][:]
        if k == 0:
            dst = g.y_acc[b][:, hh * HALF:(hh + 1) * HALF]
            srcv = pv
        elif k == 1:
            dst = g.y_acc[b][:, (1 - hh) * HALF:(2 - hh) * HALF]
            srcv = pv[:, ::-1]
        elif k == 2:
            # ys[i], i=a*48+b_ -> l = b_*48+a ; half hh: a in [24hh,24hh+24)
            dst = g.y_acc[b][:].rearrange("p (bb a) -> p bb a", a=W)[
                :, :, 24 * hh:24 * hh + 24]
            srcv = pv.rearrange("p (a bb) -> p bb a", bb=W)
        else:
            dst = g.y_acc[b][:].rearrange("p (bb a) -> p bb a", a=W)[
                :, :, 24 * (1 - hh):24 * (1 - hh) + 24]
            srcv = pv.rearrange("p (a bb) -> p bb a", bb=W)[:, ::-1, ::-1]
        nc.vector.tensor_tensor(out=dst, in0=srcv, in1=dst, op=OP.add)


def _finish_b(g, b):
    """y_fin = y_acc * silu(z); out_partial = W_out^T @ y_fin."""
    nc, wp, fps = g.nc, g.wp, g.fps
    sz = wp.tile([ESH, L], BF16, tag="u_tmp", bufs=1)
    nc.scalar.activation(out=sz[:], in_=g.z_sl[b][:], func=AF.Silu)
    yv = wp.tile([ESH, L], BF16, tag="yv", bufs=1)
    nc.vector.tensor_tensor(out=yv[:], in0=g.y_acc[b][:], in1=sz[:],
                            op=OP.mult)
    for m in range(2):
        for (c0, nf) in MM_CHUNKS:
            po = fps.tile([128, 512], F32, tag="fps", bufs=2, name="ps_out")
            nc.tensor.matmul(po[:, :nf],
                             lhsT=g.w_out_t[:, m * 128:(m + 1) * 128],
                             rhs=yv[:, c0:c0 + nf], start=True, stop=True)
            osb = wp.tile([128, 512], F32, tag="osb", bufs=1)
            nc.scalar.activation(out=osb[:, :nf], in_=po[:, :nf], func=AF.Copy)
            nc.sync.dma_start(out=g.out_d[b, m * 128:(m + 1) * 128, c0:c0 + nf],
                              in_=osb[:, :nf])


def _r32r(a):
    """Round fp32 -> fp32r (TF32-like, 10 explicit mantissa bits)."""
    b = np.ascontiguousarray(a, np.float32).view(np.uint32)
    return (((b.astype(np.uint64) + 0x1000) & 0xFFFFE000)
            .astype(np.uint32).view(np.float32))


def _bf16(a):
    return np.ascontiguousarray(np.asarray(a, np.float32)).astype(
        ml_dtypes.bfloat16)


def _host_prep(inputs):
    x = np.asarray(inputs["x"], np.float32)
    W_pos = np.asarray(inputs["W_pos"], np.float32)
    b_pos = np.asarray(inputs["b_pos"], np.float32)
    W_in = np.asarray(inputs["W_in"], np.float32)
    pw1_w = np.asarray(inputs["pw1_w"], np.float32)
    pw1_b = np.asarray(inputs["pw1_b"], np.float32)
    dw_w = np.asarray(inputs["dw_w"], np.float32)
    pw2_w = np.asarray(inputs["pw2_w"], np.float32)
    W_xproj = np.asarray(inputs["W_xproj"], np.float32)
    W_dt = np.asarray(inputs["W_dt"], np.float32)
    b_dt = np.asarray(inputs["b_dt"], np.float32)
    A_log = np.asarray(inputs["A_log"], np.float32)
    Dp = np.asarray(inputs["Dp"], np.float32)
    dir_emb = np.asarray(inputs["dir_emb"], np.float32)
    W_out = np.asarray(inputs["W_out"], np.float32)

    gy, gx = np.meshgrid(np.arange(H, dtype=np.float32),
                         np.arange(W, dtype=np.float32), indexing="ij")
    coords = np.stack([gy, gx], -1) / (H - 1) * 2 - 1
    pos = (coords.reshape(L, 2) @ W_pos + b_pos).astype(np.float32)

    common = {
        "xT": _bf16(x.transpose(0, 2, 1)),
        "posT": _bf16(pos.T),
        "w_pw1": _bf16(pw1_w.reshape(MID, E).T),
        "pw1b": np.ascontiguousarray(pw1_b.reshape(MID, 1)),
        "dwtap": np.ascontiguousarray(dw_w.reshape(MID, 9)),
    }
    w_pw2_base = pw2_w.reshape(E, MID).T  # (MID, E)
    A = -np.exp(A_log)  # (E, N)

    sel = np.zeros((2 * ESH, ESH), np.float32)
    for p in range(2 * ESH):
        sel[p, p % ESH] = 1.0
    sel = sel.astype(ml_dtypes.bfloat16)

    in_maps = []
    for c in range(NCORES):
        e0 = c * ESH
        sl = slice(e0, e0 + ESH)
        A_sl = A[sl]  # (64, 16)
        ascale = np.empty((2 * ESH, NJ), np.float32)
        for p in range(2 * ESH):
            for j in range(NJ):
                ascale[p, j] = A_sl[p % ESH, 2 * j + p // ESH]
        m = dict(common)
        # channel permutation putting this core's slice at rows [0:64]
        perm = np.concatenate([np.arange(e0, e0 + ESH),
                               np.arange(0, e0),
                               np.arange(e0 + ESH, E)])
        m["w_pw2"] = _bf16(w_pw2_base[:, perm])
        m["w_xp"] = _bf16(np.concatenate(
            [W_xproj[perm, :], np.zeros((E, 64 - (R + 2 * N)), np.float32)],
            axis=1))
        m["w_in"] = _bf16(
            np.concatenate([W_in[:, :E], W_in[:, E + e0:E + e0 + ESH]], axis=1))
        m["w_dt"] = _bf16(W_dt[:, sl])
        m["spb"] = np.ascontiguousarray((2.0 * b_dt[sl]).reshape(ESH, 1))
        m["ascale"] = ascale
        m["dire"] = np.ascontiguousarray(dir_emb[:, sl].T)
        m["dp4"] = np.ascontiguousarray((4.0 * Dp[sl]).reshape(ESH, 1))
        m["dpb"] = np.ascontiguousarray(
            (Dp[sl] * dir_emb[:, sl].sum(0)).reshape(ESH, 1))
        m["w_out"] = _bf16(W_out[sl, :])
        m["sel"] = sel
        in_maps.append(m)
    return in_maps


_PROGRAM = None
_LAST_RESULTS = None
_LAST_INSTS = None


def _get_program():
    global _PROGRAM
    if _PROGRAM is None:
        _PROGRAM = build_program()
    return _PROGRAM


def kernel(**inputs):
    global _LAST_EXEC_NS, _LAST_RESULTS
    assert int(inputs["H"]) == H and int(inputs["W"]) == W
    in_maps = _host_prep(inputs)
    if TRACE:
        _install_profile_shim()
    res = run_bass_kernel_spmd(_get_program(), in_maps,
                               list(range(NCORES)), trace=TRACE)
    _LAST_EXEC_NS = res.exec_time_ns
    _LAST_RESULTS = res.results
    global _LAST_INSTS
    _LAST_INSTS = res.instructions_and_trace
    out = np.zeros((B, DM, L), np.float32)
    for r in res.results:
        out += np.asarray(r["out"], np.float32)
    return np.ascontiguousarray(out.transpose(0, 2, 1))


# revision 14
# speedup vs baseline: 1.2624x; 1.0357x over previous
"""Trainium2 Bass kernel for the LIDAR2D 4-direction selective-scan block.

Sharding: d_inner (E=512) split 8 ways (64 channels/core). The front
(matmuls + BottConv + projections) is cheap and duplicated on every core;
each core then runs the 4-direction selective scan for its 64 channels x
16 states and emits a partial out-projection (B, Dm, L). The host sums the
8 partials and transposes back to (B, L, Dm).

Scan layout per core: tiles of [128 partitions = (nsub in {0,1}) x (64
channels), free = L] — 8 tiles j=0..7 cover states n = 2j + nsub. The
recurrence h_t = exp(A*delta_t)*h_{t-1} + delta_t*B_t*u_t runs in a single
DVE tensor_tensor_scan per tile (2 cyc/elem — the hardware floor and the
kernel's critical path). y_t = C_t . h_t is a PE matmul with a 0/1
selection matrix contracting the two nsub rows per channel, accumulated
over j in PSUM. Direction permutations (reverse / spatial transpose) are
pure access-pattern tricks on reads/writes.

Perf structure: the front runs in bf16 (1 PE pass/col) with per-512-chunk
transient tiles so its SBUF footprint is small enough to coexist with the
scan working set. Emission order overlaps front(b=1) compute under
scan(b=0)'s DVE stream: front(0); stageA(1); scan(0,k=0); conv+stageB(1);
scan(0,k=1..3); scan(1). The out-projection runs in fp32r.
"""

import os
import sys

for _p in ("/opt/trn_rl_repo", os.path.expanduser("~/.axon_site/_ro/trn_rl_repo")):
    if os.path.isdir(_p) and _p not in sys.path:
        sys.path.insert(0, _p)

import numpy as np
import ml_dtypes

import concourse.bass as bass
import concourse.bacc as bacc
import concourse.mybir as mybir
from concourse.tile import TileContext
from concourse.bass_utils import run_bass_kernel_spmd

F32 = mybir.dt.float32
F32R = mybir.dt.float32r
BF16 = mybir.dt.bfloat16
AF = mybir.ActivationFunctionType
OP = mybir.AluOpType

# Problem shape (hardcoded per the harness contract).
B, L, DM, E, N, R, MID, H, W = 2, 2304, 256, 512, 16, 16, 32, 48, 48
NCORES = 8
ESH = E // NCORES          # 64 channels per core
NJ = N // 2                # 8 scan tiles per (b, k); rows = (nsub, e_loc)
HALF = L // 2              # 1152, for PSUM-sized y accumulation

TRACE = bool(os.environ.get("KERNEL_TRACE"))
_LAST_EXEC_NS = None


def _install_profile_shim():
    """Make run_bass_kernel_spmd(trace=True) work in this container:
    register the NTFF hook (antenv.axon_hooks is absent here) and stub
    the S3 artifact upload."""
    import types
    try:
        from antenv.axon_hooks import get_axon_ntff_profile_hook  # noqa: F401
    except ImportError:
        import antenv
        mod = types.ModuleType("antenv.axon_hooks")
        mod._HOOK = None
        mod.set_axon_ntff_profile_hook = lambda h: setattr(mod, "_HOOK", h)
        mod.get_axon_ntff_profile_hook = lambda: mod._HOOK
        sys.modules["antenv.axon_hooks"] = mod
        antenv.axon_hooks = mod
        try:
            from trn_agent_boot.trn_boot import _ntff_profile_via_ctypes
            hook = _ntff_profile_via_ctypes("/opt/axon/libaxon_pjrt.so")
            if hook is not None:
                mod._HOOK = hook
        except Exception as e:  # pragma: no cover
            print(f"profile shim: hook install failed: {e}")
    import concourse.bass_utils as bu
    bu.upload_artifacts = lambda tmpdir: f"file://{tmpdir}"


def _chunks(total, step):
    out = []
    c0 = 0
    while c0 < total:
        out.append((c0, min(step, total - c0)))
        c0 += step
    return out


MM_CHUNKS = _chunks(L, 512)          # matmul free-dim chunks over full L
MM_CHUNKS_HALF = _chunks(HALF, 512)  # chunks within a 1152 half


class Env:
    pass


def build_program():
    nc = bacc.Bacc()
    g = Env()
    g.nc = nc

    # ---- DRAM parameters (same shapes on every core; values differ) ----
    g.xT_d = nc.declare_dram_parameter("xT", [B, DM, L], BF16, isOutput=False)
    g.w_in_d = nc.declare_dram_parameter("w_in", [DM, E + ESH], BF16, isOutput=False)
    g.w_pw1_d = nc.declare_dram_parameter("w_pw1", [E, MID], BF16, isOutput=False)
    g.pw1b_d = nc.declare_dram_parameter("pw1b", [MID, 1], F32, isOutput=False)
    g.dwtap_d = nc.declare_dram_parameter("dwtap", [MID, 9], F32, isOutput=False)
    g.w_pw2_d = nc.declare_dram_parameter("w_pw2", [MID, E], BF16, isOutput=False)
    g.w_xp_d = nc.declare_dram_parameter("w_xp", [E, 64], BF16, isOutput=False)
    g.w_dt_d = nc.declare_dram_parameter("w_dt", [R, ESH], BF16, isOutput=False)
    g.spb_d = nc.declare_dram_parameter("spb", [ESH, 1], F32, isOutput=False)
    g.ascale_d = nc.declare_dram_parameter("ascale", [2 * ESH, NJ], F32, isOutput=False)
    g.dire_d = nc.declare_dram_parameter("dire", [ESH, 4], F32, isOutput=False)
    g.dp4_d = nc.declare_dram_parameter("dp4", [ESH, 1], F32, isOutput=False)
    g.dpb_d = nc.declare_dram_parameter("dpb", [ESH, 1], F32, isOutput=False)
    g.w_out_d = nc.declare_dram_parameter("w_out", [ESH, DM], BF16, isOutput=False)
    g.sel_d = nc.declare_dram_parameter("sel", [2 * ESH, ESH], BF16, isOutput=False)
    g.out_d = nc.declare_dram_parameter("out", [B, DM, L], F32, isOutput=True)

    with TileContext(nc) as tc:
        g.tc = tc
        with tc.tile_pool(name="const", bufs=1) as cp, \
             tc.tile_pool(name="persist", bufs=1) as pp, \
             tc.tile_pool(name="front", bufs=1) as fp, \
             tc.tile_pool(name="scan", bufs=1) as sp, \
             tc.tile_pool(name="work", bufs=2) as wp, \
             tc.tile_pool(name="fpsum", bufs=2, space="PSUM") as fps, \
             tc.tile_pool(name="ypsum", bufs=1, space="PSUM") as yps, \
             tc.tile_pool(name="bounce", bufs=1, space="DRAM") as bp:
            g.cp, g.pp, g.fp, g.sp, g.wp, g.fps, g.yps, g.bp = \
                cp, pp, fp, sp, wp, fps, yps, bp
            _load_consts(g)

            # persistent per-b products of the front
            g.xc_sl = [pp.tile([ESH, L], BF16, tag=f"xc_sl{b}", name=f"xc_sl{b}") for b in range(B)]
            g.delta_rep = [pp.tile([128, L], F32, tag=f"drep{b}", name=f"drep{b}") for b in range(B)]
            g.z_sl = [pp.tile([ESH, L], BF16, tag=f"z{b}", name=f"z{b}") for b in range(B)]
            g.y_acc = [pp.tile([ESH, L], F32, tag=f"yacc{b}", name=f"yacc{b}") for b in range(B)]
            g.bsrc = [bp.tile([N, L], BF16, tag=f"bsrc{b}", name=f"bsrc{b}") for b in range(B)]
            g.csrc = [bp.tile([N, L], BF16, tag=f"csrc{b}", name=f"csrc{b}") for b in range(B)]
            # h12/acc full-L per-b (conv needs the whole plane)
            g.h12 = fp.tile([MID, L], BF16, tag="h12", name="h12")
            g.accB = fp.tile([MID, L], BF16, tag="accB", name="accB")
            g.xdbl = fp.tile([64, L], BF16, tag="xdbl", name="xdbl")
            g.dtlin = fp.tile([ESH, L], BF16, tag="dtlin", name="dtlin")
            g.dA = [None] * NJ
            g.du = [None] * 4

            # ---- emission schedule (overlap front(1) under scan(0)) ----
            _front_stageA(g, 0)
            _front_conv(g, 0)
            _front_stageB(g, 0)
            _front_stageA(g, 1)
            _scan_prep(g, 0)
            _scan_k(g, 0, 0)
            _front_conv(g, 1)
            _front_stageB(g, 1)
            for k in range(1, 4):
                _scan_k(g, 0, k)
            _finish_b(g, 0)
            _scan_prep(g, 1)
            for k in range(4):
                _scan_k(g, 1, k)
            _finish_b(g, 1)

    nc.finalize()
    return nc


def _load_consts(g):
    nc, cp = g.nc, g.cp
    g.w_in_t = [cp.tile([128, E + ESH], BF16, tag=f"w_in{t}", name=f"w_in{t}") for t in range(2)]
    for t in range(2):
        nc.sync.dma_start(out=g.w_in_t[t][:], in_=g.w_in_d[t * 128:(t + 1) * 128, :])
    g.w_pw1_t = [cp.tile([128, MID], BF16, tag=f"w_pw1{t}", name=f"w_pw1{t}") for t in range(4)]
    for t in range(4):
        nc.sync.dma_start(out=g.w_pw1_t[t][:], in_=g.w_pw1_d[t * 128:(t + 1) * 128, :])
    g.pw1b_t = cp.tile([MID, 1], F32, tag="pw1b", name="pw1b")
    nc.sync.dma_start(out=g.pw1b_t[:], in_=g.pw1b_d[:])
    g.dwtap_t = cp.tile([MID, 9], F32, tag="dwtap", name="dwtap")
    nc.sync.dma_start(out=g.dwtap_t[:], in_=g.dwtap_d[:])
    g.w_pw2_t = cp.tile([MID, E], BF16, tag="w_pw2", name="w_pw2")
    nc.sync.dma_start(out=g.w_pw2_t[:], in_=g.w_pw2_d[:])
    g.w_xp_t = [cp.tile([128, 64], BF16, tag=f"w_xp{t}", name=f"w_xp{t}") for t in range(4)]
    for t in range(4):
        nc.sync.dma_start(out=g.w_xp_t[t][:], in_=g.w_xp_d[t * 128:(t + 1) * 128, :])
    g.w_dt_t = cp.tile([R, ESH], BF16, tag="w_dt", name="w_dt")
    nc.sync.dma_start(out=g.w_dt_t[:], in_=g.w_dt_d[:])
    g.spb_t = cp.tile([ESH, 1], F32, tag="spb", name="spb")
    nc.sync.dma_start(out=g.spb_t[:], in_=g.spb_d[:])
    g.ascale_t = cp.tile([2 * ESH, NJ], F32, tag="ascale", name="ascale")
    nc.sync.dma_start(out=g.ascale_t[:], in_=g.ascale_d[:])
    g.dire_t = cp.tile([ESH, 4], F32, tag="dire", name="dire")
    nc.sync.dma_start(out=g.dire_t[:], in_=g.dire_d[:])
    g.dp4_t = cp.tile([ESH, 1], F32, tag="dp4", name="dp4")
    nc.sync.dma_start(out=g.dp4_t[:], in_=g.dp4_d[:])
    g.dpb_t = cp.tile([ESH, 1], F32, tag="dpb", name="dpb")
    nc.sync.dma_start(out=g.dpb_t[:], in_=g.dpb_d[:])
    g.w_out_t = cp.tile([ESH, DM], BF16, tag="w_out", name="w_out")
    nc.sync.dma_start(out=g.w_out_t[:], in_=g.w_out_d[:])
    g.sel_t = cp.tile([2 * ESH, ESH], BF16, tag="sel", name="sel")
    nc.sync.dma_start(out=g.sel_t[:], in_=g.sel_d[:])


def _front_stageA(g, b):
    """x+pos -> xz -> (z slice, xh) -> pw1 -> h12 (full [MID, L])."""
    nc, fp, fps = g.nc, g.fp, g.fps
    for (c0, nf) in MM_CHUNKS:
        xin = [fp.tile([128, 512], BF16, tag=f"xin{t}", bufs=2, name=f"xin{t}") for t in range(2)]
        for t in range(2):
            nc.sync.dma_start(out=xin[t][:, :nf],
                              in_=g.xT_d[b, t * 128:(t + 1) * 128, c0:c0 + nf])
        xhc = [fp.tile([128, 512], BF16, tag=f"xhc{m}", bufs=2, name=f"xhc{m}") for m in range(4)]
        for m in range(4):
            ps = fps.tile([128, 512], F32, tag="fps", bufs=2, name="ps_xz")
            for kt in range(2):
                nc.tensor.matmul(ps[:, :nf],
                                 lhsT=g.w_in_t[kt][:, m * 128:(m + 1) * 128],
                                 rhs=xin[kt][:, :nf],
                                 start=(kt == 0), stop=(kt == 1))
            nc.scalar.activation(out=xhc[m][:, :nf], in_=ps[:, :nf], func=AF.Copy)
        psz = fps.tile([ESH, 512], F32, tag="fps", bufs=2, name="ps_z")
        for kt in range(2):
            nc.tensor.matmul(psz[:, :nf],
                             lhsT=g.w_in_t[kt][:, E:E + ESH],
                             rhs=xin[kt][:, :nf],
                             start=(kt == 0), stop=(kt == 1))
        nc.scalar.activation(out=g.z_sl[b][:, c0:c0 + nf], in_=psz[:, :nf],
                             func=AF.Copy)
        ps1 = fps.tile([MID, 512], F32, tag="fps", bufs=2, name="ps_pw1")
        for kt in range(4):
            nc.tensor.matmul(ps1[:, :nf], lhsT=g.w_pw1_t[kt][:],
                             rhs=xhc[kt][:, :nf],
                             start=(kt == 0), stop=(kt == 3))
        nc.scalar.activation(out=g.h12[:, c0:c0 + nf], in_=ps1[:, :nf],
                             func=AF.Identity, bias=g.pw1b_t[:])


def _front_conv(g, b):
    """Depthwise 3x3 on h12 -> accB (bf16 copy for pw2 rhs)."""
    nc, fp = g.nc, g.fp
    acc = fp.tile([MID, L], F32, tag="dwacc")
    acc3 = acc[:].rearrange("p (h w) -> p h w", w=W)
    h3 = g.h12[:].rearrange("p (h w) -> p h w", w=W)
    nc.vector.tensor_scalar(out=acc[:], in0=g.h12[:],
                            scalar1=g.dwtap_t[:, 4:5], scalar2=None,
                            op0=OP.mult)
    for ky in range(3):
        for kx in range(3):
            if ky == 1 and kx == 1:
                continue
            dy, dx = ky - 1, kx - 1
            h0, h1 = max(0, -dy), H - max(0, dy)
            w0, w1 = max(0, -dx), W - max(0, dx)
            nc.vector.scalar_tensor_tensor(
                out=acc3[:, h0:h1, w0:w1],
                in0=h3[:, h0 + dy:h1 + dy, w0 + dx:w1 + dx],
                scalar=g.dwtap_t[:, ky * 3 + kx:ky * 3 + kx + 1],
                in1=acc3[:, h0:h1, w0:w1],
                op0=OP.mult, op1=OP.add)
    nc.scalar.activation(out=g.accB[:], in_=acc[:], func=AF.Copy)


def _front_stageB(g, b):
    """pw2+SiLU -> xc chunks -> (xc_sl slice, x_dbl, delta, B/C to DRAM)."""
    nc, fp, fps = g.nc, g.fp, g.fps
    for (c0, nf) in MM_CHUNKS:
        xcc = [fp.tile([128, 512], BF16, tag=f"xcc{m}", bufs=2, name=f"xcc{m}") for m in range(4)]
        for m in range(4):
            ps2 = fps.tile([128, 512], F32, tag="fps", bufs=2, name="ps_pw2")
            nc.tensor.matmul(ps2[:, :nf],
                             lhsT=g.w_pw2_t[:, m * 128:(m + 1) * 128],
                             rhs=g.accB[:, c0:c0 + nf], start=True, stop=True)
            nc.scalar.activation(out=xcc[m][:, :nf], in_=ps2[:, :nf],
                                 func=AF.Silu)
        nc.sync.dma_start(out=g.xc_sl[b][:, c0:c0 + nf], in_=xcc[0][0:ESH, :nf])
        ps3 = fps.tile([64, 512], F32, tag="fps", bufs=2, name="ps_xdbl")
        for kt in range(4):
            nc.tensor.matmul(ps3[:, :nf], lhsT=g.w_xp_t[kt][:],
                             rhs=xcc[kt][:, :nf],
                             start=(kt == 0), stop=(kt == 3))
        nc.scalar.activation(out=g.xdbl[:, c0:c0 + nf], in_=ps3[:, :nf],
                             func=AF.Copy)
        ps4 = fps.tile([ESH, 512], F32, tag="fps", bufs=2, name="ps_dt")
        nc.tensor.matmul(ps4[:, :nf], lhsT=g.w_dt_t[:],
                         rhs=g.xdbl[0:R, c0:c0 + nf], start=True, stop=True)
        nc.scalar.activation(out=g.dtlin[:, c0:c0 + nf], in_=ps4[:, :nf],
                             func=AF.Copy)
    # softplus(v) = ln(1 + exp(v)); |v| < ~10 here so exp cannot overflow.
    # Batched over full L to avoid ACT table reloads inside the chunk loop.
    nc.scalar.activation(out=g.delta_rep[b][0:ESH, :], in_=g.dtlin[:],
                         func=AF.Exp, bias=g.spb_t[:])
    nc.scalar.activation(out=g.delta_rep[b][0:ESH, :],
                         in_=g.delta_rep[b][0:ESH, :], func=AF.Ln, bias=1.0)
    # B/C rows to DRAM for later partition-broadcast loads
    nc.sync.dma_start(out=g.bsrc[b][:], in_=g.xdbl[R:R + N, :])
    nc.sync.dma_start(out=g.csrc[b][:], in_=g.xdbl[R + N:R + 2 * N, :])
    # duplicate delta rows [0:64] -> [64:128]
    nc.sync.dma_start(out=g.delta_rep[b][ESH:2 * ESH, :],
                      in_=g.delta_rep[b][0:ESH, :])


def _scan_prep(g, b):
    """y_acc init (D*u skip), dA exps, per-direction du tiles."""
    nc, sp, wp = g.nc, g.sp, g.wp
    nc.scalar.activation(out=g.y_acc[b][:], in_=g.xc_sl[b][:],
                         func=AF.Identity, bias=g.dpb_t[:], scale=g.dp4_t[:])
    for j in range(NJ):
        g.dA[j] = sp.tile([128, L], BF16, tag=f"dA{j}", name=f"dA{j}")
        nc.scalar.activation(out=g.dA[j][:], in_=g.delta_rep[b][:],
                             func=AF.Exp, scale=g.ascale_t[:, j:j + 1])
    xc3 = g.xc_sl[b][:].rearrange("p (h w) -> p h w", w=W)
    xcT = g.xc_sl[b][:].rearrange("p (h w) -> p w h", w=W)
    for k in range(4):
        src = [xc3, xc3[:, ::-1, ::-1], xcT, xcT[:, ::-1, ::-1]][k]
        u_tmp = wp.tile([ESH, L], BF16, tag="u_tmp", bufs=1)
        u3 = u_tmp[:].rearrange("p (a c) -> p a c", c=W)
        nc.scalar.activation(out=u3, in_=src, func=AF.Identity,
                             bias=g.dire_t[:, k:k + 1])
        g.du[k] = sp.tile([128, L], BF16, tag=f"du{k}", name=f"du{k}")
        nc.vector.tensor_tensor(out=g.du[k][0:ESH, :],
                                in0=g.delta_rep[b][0:ESH, :],
                                in1=u_tmp[:], op=OP.mult)
        nc.sync.dma_start(out=g.du[k][ESH:2 * ESH, :], in_=g.du[k][0:ESH, :])


def _scan_k(g, b, k):
    """One direction's 8 scan tiles + y accumulation."""
    nc, wp, yps = g.nc, g.wp, g.yps
    ypsum = [yps.tile([ESH, HALF], F32, tag=f"yps{h}", name=f"yps{h}") for h in range(2)]
    for j in range(NJ):
        B_t = wp.tile([128, L], BF16, tag="B_t", bufs=2)
        C_t = wp.tile([128, L], BF16, tag="C_t", bufs=2)
        for ns in range(2):
            nc.sync.dma_start(
                out=B_t[ns * ESH:(ns + 1) * ESH, :],
                in_=g.bsrc[b][2 * j + ns:2 * j + ns + 1, :]
                .to_broadcast((ESH, L)))
            nc.sync.dma_start(
                out=C_t[ns * ESH:(ns + 1) * ESH, :],
                in_=g.csrc[b][2 * j + ns:2 * j + ns + 1, :]
                .to_broadcast((ESH, L)))
        dbu = wp.tile([128, L], BF16, tag="workA", bufs=2)
        nc.vector.tensor_tensor(out=dbu[:], in0=g.du[k][:], in1=B_t[:],
                                op=OP.mult)
        h_t = wp.tile([128, L], BF16, tag="workH", bufs=2)
        nc.vector.tensor_tensor_scan(out=h_t[:], data0=g.dA[j][:],
                                     data1=dbu[:], initial=0.0,
                                     op0=OP.mult, op1=OP.add)
        hc = wp.tile([128, L], BF16, tag="workA", bufs=2)
        nc.vector.tensor_tensor(out=hc[:], in0=h_t[:], in1=C_t[:], op=OP.mult)
        for hh in range(2):
            for (c0, nf) in MM_CHUNKS_HALF:
                nc.tensor.matmul(
                    ypsum[hh][:, c0:c0 + nf],
                    lhsT=g.sel_t[:],
                    rhs=hc[:, hh * HALF + c0:hh * HALF + c0 + nf],
                    start=(j == 0), stop=(j == NJ - 1))
    # accumulate un-permuted ys_k into y_acc
    for hh in range(2):
        pv = ypsum[hh][:]
        if k == 0:
            dst = g.y_acc[b][:, hh * HALF:(hh + 1) * HALF]
            srcv = pv
        elif k == 1:
            dst = g.y_acc[b][:, (1 - hh) * HALF:(2 - hh) * HALF]
            srcv = pv[:, ::-1]
        elif k == 2:
            # ys[i], i=a*48+b_ -> l = b_*48+a ; half hh: a in [24hh,24hh+24)
            dst = g.y_acc[b][:].rearrange("p (bb a) -> p bb a", a=W)[
                :, :, 24 * hh:24 * hh + 24]
            srcv = pv.rearrange("p (a bb) -> p bb a", bb=W)
        else:
            dst = g.y_acc[b][:].rearrange("p (bb a) -> p bb a", a=W)[
                :, :, 24 * (1 - hh):24 * (1 - hh) + 24]
            srcv = pv.rearrange("p (a bb) -> p bb a", bb=W)[:, ::-1, ::-1]
        nc.vector.tensor_tensor(out=dst, in0=srcv, in1=dst, op=OP.add)


def _finish_b(g, b):
    """y_fin = y_acc * silu(z); out_partial = W_out^T @ y_fin."""
    nc, wp, fps = g.nc, g.wp, g.fps
    sz = wp.tile([ESH, L], BF16, tag="u_tmp", bufs=1)
    nc.scalar.activation(out=sz[:], in_=g.z_sl[b][:], func=AF.Silu)
    yv = wp.tile([ESH, L], BF16, tag="yv", bufs=1)
    nc.vector.tensor_tensor(out=yv[:], in0=g.y_acc[b][:], in1=sz[:],
                            op=OP.mult)
    for m in range(2):
        for (c0, nf) in MM_CHUNKS:
            po = fps.tile([128, 512], F32, tag="fps", bufs=2, name="ps_out")
            nc.tensor.matmul(po[:, :nf],
                             lhsT=g.w_out_t[:, m * 128:(m + 1) * 128],
                             rhs=yv[:, c0:c0 + nf], start=True, stop=True)
            osb = wp.tile([128, 512], F32, tag="osb", bufs=1)
            nc.scalar.activation(out=osb[:, :nf], in_=po[:, :nf], func=AF.Copy)
            nc.sync.dma_start(out=g.out_d[b, m * 128:(m + 1) * 128, c0:c0 + nf],
                              in_=osb[:, :nf])


def _r32r(a):
    """Round fp32 -> fp32r (TF32-like, 10 explicit mantissa bits)."""
    b = np.ascontiguousarray(a, np.float32).view(np.uint32)
    return (((b.astype(np.uint64) + 0x1000) & 0xFFFFE000)
            .astype(np.uint32).view(np.float32))


def _bf16(a):
    return np.ascontiguousarray(np.asarray(a, np.float32)).astype(
        ml_dtypes.bfloat16)


def _host_prep(inputs):
    x = np.asarray(inputs["x"], np.float32)
    W_pos = np.asarray(inputs["W_pos"], np.float32)
    b_pos = np.asarray(inputs["b_pos"], np.float32)
    W_in = np.asarray(inputs["W_in"], np.float32)
    pw1_w = np.asarray(inputs["pw1_w"], np.float32)
    pw1_b = np.asarray(inputs["pw1_b"], np.float32)
    dw_w = np.asarray(inputs["dw_w"], np.float32)
    pw2_w = np.asarray(inputs["pw2_w"], np.float32)
    W_xproj = np.asarray(inputs["W_xproj"], np.float32)
    W_dt = np.asarray(inputs["W_dt"], np.float32)
    b_dt = np.asarray(inputs["b_dt"], np.float32)
    A_log = np.asarray(inputs["A_log"], np.float32)
    Dp = np.asarray(inputs["Dp"], np.float32)
    dir_emb = np.asarray(inputs["dir_emb"], np.float32)
    W_out = np.asarray(inputs["W_out"], np.float32)

    gy, gx = np.meshgrid(np.arange(H, dtype=np.float32),
                         np.arange(W, dtype=np.float32), indexing="ij")
    coords = np.stack([gy, gx], -1) / (H - 1) * 2 - 1
    pos = (coords.reshape(L, 2) @ W_pos + b_pos).astype(np.float32)

    common = {
        "xT": _bf16((x + pos[None]).transpose(0, 2, 1)),
        "w_pw1": _bf16(pw1_w.reshape(MID, E).T),
        "pw1b": np.ascontiguousarray(pw1_b.reshape(MID, 1)),
        "dwtap": np.ascontiguousarray(dw_w.reshape(MID, 9)),
    }
    w_pw2_base = pw2_w.reshape(E, MID).T  # (MID, E)
    A = -np.exp(A_log)  # (E, N)

    sel = np.zeros((2 * ESH, ESH), np.float32)
    for p in range(2 * ESH):
        sel[p, p % ESH] = 1.0
    sel = sel.astype(ml_dtypes.bfloat16)

    in_maps = []
    for c in range(NCORES):
        e0 = c * ESH
        sl = slice(e0, e0 + ESH)
        A_sl = A[sl]  # (64, 16)
        ascale = np.empty((2 * ESH, NJ), np.float32)
        for p in range(2 * ESH):
            for j in range(NJ):
                ascale[p, j] = A_sl[p % ESH, 2 * j + p // ESH]
        m = dict(common)
        # channel permutation putting this core's slice at rows [0:64]
        perm = np.concatenate([np.arange(e0, e0 + ESH),
                               np.arange(0, e0),
                               np.arange(e0 + ESH, E)])
        m["w_pw2"] = _bf16(w_pw2_base[:, perm])
        m["w_xp"] = _bf16(np.concatenate(
            [W_xproj[perm, :], np.zeros((E, 64 - (R + 2 * N)), np.float32)],
            axis=1))
        m["w_in"] = _bf16(
            np.concatenate([W_in[:, :E], W_in[:, E + e0:E + e0 + ESH]], axis=1))
        m["w_dt"] = _bf16(W_dt[:, sl])
        m["spb"] = np.ascontiguousarray((2.0 * b_dt[sl]).reshape(ESH, 1))
        m["ascale"] = ascale
        m["dire"] = np.ascontiguousarray(dir_emb[:, sl].T)
        m["dp4"] = np.ascontiguousarray((4.0 * Dp[sl]).reshape(ESH, 1))
        m["dpb"] = np.ascontiguousarray(
            (Dp[sl] * dir_emb[:, sl].sum(0)).reshape(ESH, 1))
        m["w_out"] = _bf16(W_out[sl, :])
        m["sel"] = sel
        in_maps.append(m)
    return in_maps


_PROGRAM = None
_LAST_RESULTS = None
_LAST_INSTS = None


def _get_program():
    global _PROGRAM
    if _PROGRAM is None:
        _PROGRAM = build_program()
    return _PROGRAM


def kernel(**inputs):
    global _LAST_EXEC_NS, _LAST_RESULTS
    assert int(inputs["H"]) == H and int(inputs["W"]) == W
    in_maps = _host_prep(inputs)
    if TRACE:
        _install_profile_shim()
    res = run_bass_kernel_spmd(_get_program(), in_maps,
                               list(range(NCORES)), trace=TRACE)
    _LAST_EXEC_NS = res.exec_time_ns
    _LAST_RESULTS = res.results
    global _LAST_INSTS
    _LAST_INSTS = res.instructions_and_trace
    out = np.zeros((B, DM, L), np.float32)
    for r in res.results:
        out += np.asarray(r["out"], np.float32)
    return np.ascontiguousarray(out.transpose(0, 2, 1))


# revision 16
# speedup vs baseline: 1.2656x; 1.0026x over previous
"""Trainium2 Bass kernel for the LIDAR2D 4-direction selective-scan block.

Sharding: d_inner (E=512) split 8 ways (64 channels/core). The front
(matmuls + BottConv + projections) is cheap and duplicated on every core;
each core then runs the 4-direction selective scan for its 64 channels x
16 states and emits a partial out-projection (B, Dm, L). The host sums the
8 partials and transposes back to (B, L, Dm).

Scan layout per core: tiles of [128 partitions = (nsub in {0,1}) x (64
channels), free = L] — 8 tiles j=0..7 cover states n = 2j + nsub. The
recurrence h_t = exp(A*delta_t)*h_{t-1} + delta_t*B_t*u_t runs in a single
DVE tensor_tensor_scan per tile (2 cyc/elem — the hardware floor and the
kernel's critical path). y_t = C_t . h_t is a PE matmul with a 0/1
selection matrix contracting the two nsub rows per channel, accumulated
over j in PSUM. Direction permutations (reverse / spatial transpose) are
pure access-pattern tricks on reads/writes.

Perf structure: the front runs in bf16 (1 PE pass/col) with per-512-chunk
transient tiles so its SBUF footprint is small enough to coexist with the
scan working set. Emission order overlaps front(b=1) compute under
scan(b=0)'s DVE stream: front(0); stageA(1); scan(0,k=0); conv+stageB(1);
scan(0,k=1..3); scan(1). The out-projection runs in fp32r.
"""

import os
import sys

for _p in ("/opt/trn_rl_repo", os.path.expanduser("~/.axon_site/_ro/trn_rl_repo")):
    if os.path.isdir(_p) and _p not in sys.path:
        sys.path.insert(0, _p)

import numpy as np
import ml_dtypes

import concourse.bass as bass
import concourse.bacc as bacc
import concourse.mybir as mybir
from concourse.tile import TileContext
from concourse.bass_utils import run_bass_kernel_spmd

F32 = mybir.dt.float32
F32R = mybir.dt.float32r
BF16 = mybir.dt.bfloat16
AF = mybir.ActivationFunctionType
OP = mybir.AluOpType

# Problem shape (hardcoded per the harness contract).
B, L, DM, E, N, R, MID, H, W = 2, 2304, 256, 512, 16, 16, 32, 48, 48
NCORES = 8
ESH = E // NCORES          # 64 channels per core
NJ = N // 2                # 8 scan tiles per (b, k); rows = (nsub, e_loc)
HALF = L // 2              # 1152, for PSUM-sized y accumulation

TRACE = bool(os.environ.get("KERNEL_TRACE"))
_LAST_EXEC_NS = None


def _install_profile_shim():
    """Make run_bass_kernel_spmd(trace=True) work in this container:
    register the NTFF hook (antenv.axon_hooks is absent here) and stub
    the S3 artifact upload."""
    import types
    try:
        from antenv.axon_hooks import get_axon_ntff_profile_hook  # noqa: F401
    except ImportError:
        import antenv
        mod = types.ModuleType("antenv.axon_hooks")
        mod._HOOK = None
        mod.set_axon_ntff_profile_hook = lambda h: setattr(mod, "_HOOK", h)
        mod.get_axon_ntff_profile_hook = lambda: mod._HOOK
        sys.modules["antenv.axon_hooks"] = mod
        antenv.axon_hooks = mod
        try:
            from trn_agent_boot.trn_boot import _ntff_profile_via_ctypes
            hook = _ntff_profile_via_ctypes("/opt/axon/libaxon_pjrt.so")
            if hook is not None:
                mod._HOOK = hook
        except Exception as e:  # pragma: no cover
            print(f"profile shim: hook install failed: {e}")
    import concourse.bass_utils as bu
    bu.upload_artifacts = lambda tmpdir: f"file://{tmpdir}"


def _chunks(total, step):
    out = []
    c0 = 0
    while c0 < total:
        out.append((c0, min(step, total - c0)))
        c0 += step
    return out


MM_CHUNKS = _chunks(L, 512)          # matmul free-dim chunks over full L
MM_CHUNKS_HALF = _chunks(HALF, 512)  # chunks within a 1152 half


class Env:
    pass


def build_program():
    nc = bacc.Bacc()
    g = Env()
    g.nc = nc

    # ---- DRAM parameters (same shapes on every core; values differ) ----
    g.xT_d = nc.declare_dram_parameter("xT", [B, DM, L], BF16, isOutput=False)
    g.w_in_d = nc.declare_dram_parameter("w_in", [DM, E + ESH], BF16, isOutput=False)
    g.w_pw1_d = nc.declare_dram_parameter("w_pw1", [E, MID], BF16, isOutput=False)
    g.pw1b_d = nc.declare_dram_parameter("pw1b", [MID, 1], F32, isOutput=False)
    g.dwtap_d = nc.declare_dram_parameter("dwtap", [MID, 9], F32, isOutput=False)
    g.w_pw2_d = nc.declare_dram_parameter("w_pw2", [MID, E], BF16, isOutput=False)
    g.w_xp_d = nc.declare_dram_parameter("w_xp", [E, 64], BF16, isOutput=False)
    g.w_dt_d = nc.declare_dram_parameter("w_dt", [R, ESH], BF16, isOutput=False)
    g.spb_d = nc.declare_dram_parameter("spb", [ESH, 1], F32, isOutput=False)
    g.ascale_d = nc.declare_dram_parameter("ascale", [2 * ESH, NJ], F32, isOutput=False)
    g.dire_d = nc.declare_dram_parameter("dire", [ESH, 4], F32, isOutput=False)
    g.dp4_d = nc.declare_dram_parameter("dp4", [ESH, 1], F32, isOutput=False)
    g.dpb_d = nc.declare_dram_parameter("dpb", [ESH, 1], F32, isOutput=False)
    g.w_out_d = nc.declare_dram_parameter("w_out", [ESH, DM], BF16, isOutput=False)
    g.sel_d = nc.declare_dram_parameter("sel", [2 * ESH, ESH], BF16, isOutput=False)
    g.out_d = nc.declare_dram_parameter("out", [B, DM, L], F32, isOutput=True)

    with TileContext(nc) as tc:
        g.tc = tc
        with tc.tile_pool(name="const", bufs=1) as cp, \
             tc.tile_pool(name="persist", bufs=1) as pp, \
             tc.tile_pool(name="front", bufs=1) as fp, \
             tc.tile_pool(name="scan", bufs=1) as sp, \
             tc.tile_pool(name="work", bufs=2) as wp, \
             tc.tile_pool(name="fpsum", bufs=2, space="PSUM") as fps, \
             tc.tile_pool(name="ypsum", bufs=1, space="PSUM") as yps, \
             tc.tile_pool(name="bounce", bufs=1, space="DRAM") as bp:
            g.cp, g.pp, g.fp, g.sp, g.wp, g.fps, g.yps, g.bp = \
                cp, pp, fp, sp, wp, fps, yps, bp
            _load_consts(g)

            # persistent per-b products of the front
            g.xc_sl = [pp.tile([ESH, L], BF16, tag=f"xc_sl{b}", name=f"xc_sl{b}") for b in range(B)]
            g.delta_rep = [pp.tile([128, L], F32, tag=f"drep{b}", name=f"drep{b}") for b in range(B)]
            g.z_sl = [pp.tile([ESH, L], BF16, tag=f"z{b}", name=f"z{b}") for b in range(B)]
            g.y_acc = [pp.tile([ESH, L], F32, tag=f"yacc{b}", name=f"yacc{b}") for b in range(B)]
            g.bsrc = [bp.tile([N, L], BF16, tag=f"bsrc{b}", name=f"bsrc{b}") for b in range(B)]
            g.csrc = [bp.tile([N, L], BF16, tag=f"csrc{b}", name=f"csrc{b}") for b in range(B)]
            # h12/acc full-L per-b (conv needs the whole plane)
            g.h12 = fp.tile([MID, L], BF16, tag="h12", name="h12")
            g.accB = fp.tile([MID, L], BF16, tag="accB", name="accB")
            g.xdbl = fp.tile([64, L], BF16, tag="xdbl", name="xdbl")
            g.dtlin = fp.tile([ESH, L], BF16, tag="dtlin", name="dtlin")
            g.dA = [None] * NJ
            g.du = [None] * 4

            # ---- emission schedule (overlap front(1) under scan(0)) ----
            _front_stageA(g, 0)
            _front_conv(g, 0)
            _front_stageB(g, 0)
            _front_stageA(g, 1)
            _scan_prep(g, 0)
            _scan_k(g, 0, 0)
            _scan_k(g, 0, 1)
            _front_conv(g, 1)
            _front_stageB(g, 1)
            for k in range(2, 4):
                _scan_k(g, 0, k)
            _finish_b(g, 0)
            _scan_prep(g, 1)
            for k in range(4):
                _scan_k(g, 1, k)
            _finish_b(g, 1)

    nc.finalize()
    return nc


def _load_consts(g):
    nc, cp = g.nc, g.cp
    g.w_in_t = [cp.tile([128, E + ESH], BF16, tag=f"w_in{t}", name=f"w_in{t}") for t in range(2)]
    for t in range(2):
        nc.sync.dma_start(out=g.w_in_t[t][:], in_=g.w_in_d[t * 128:(t + 1) * 128, :])
    g.w_pw1_t = [cp.tile([128, MID], BF16, tag=f"w_pw1{t}", name=f"w_pw1{t}") for t in range(4)]
    for t in range(4):
        nc.sync.dma_start(out=g.w_pw1_t[t][:], in_=g.w_pw1_d[t * 128:(t + 1) * 128, :])
    g.pw1b_t = cp.tile([MID, 1], F32, tag="pw1b", name="pw1b")
    nc.sync.dma_start(out=g.pw1b_t[:], in_=g.pw1b_d[:])
    g.dwtap_t = cp.tile([MID, 9], F32, tag="dwtap", name="dwtap")
    nc.sync.dma_start(out=g.dwtap_t[:], in_=g.dwtap_d[:])
    g.w_pw2_t = cp.tile([MID, E], BF16, tag="w_pw2", name="w_pw2")
    nc.sync.dma_start(out=g.w_pw2_t[:], in_=g.w_pw2_d[:])
    g.w_xp_t = [cp.tile([128, 64], BF16, tag=f"w_xp{t}", name=f"w_xp{t}") for t in range(4)]
    for t in range(4):
        nc.sync.dma_start(out=g.w_xp_t[t][:], in_=g.w_xp_d[t * 128:(t + 1) * 128, :])
    g.w_dt_t = cp.tile([R, ESH], BF16, tag="w_dt", name="w_dt")
    nc.sync.dma_start(out=g.w_dt_t[:], in_=g.w_dt_d[:])
    g.spb_t = cp.tile([ESH, 1], F32, tag="spb", name="spb")
    nc.sync.dma_start(out=g.spb_t[:], in_=g.spb_d[:])
    g.ascale_t = cp.tile([2 * ESH, NJ], F32, tag="ascale", name="ascale")
    nc.sync.dma_start(out=g.ascale_t[:], in_=g.ascale_d[:])
    g.dire_t = cp.tile([ESH, 4], F32, tag="dire", name="dire")
    nc.sync.dma_start(out=g.dire_t[:], in_=g.dire_d[:])
    g.dp4_t = cp.tile([ESH, 1], F32, tag="dp4", name="dp4")
    nc.sync.dma_start(out=g.dp4_t[:], in_=g.dp4_d[:])
    g.dpb_t = cp.tile([ESH, 1], F32, tag="dpb", name="dpb")
    nc.sync.dma_start(out=g.dpb_t[:], in_=g.dpb_d[:])
    g.w_out_t = cp.tile([ESH, DM], BF16, tag="w_out", name="w_out")
    nc.sync.dma_start(out=g.w_out_t[:], in_=g.w_out_d[:])
    g.sel_t = cp.tile([2 * ESH, ESH], BF16, tag="sel", name="sel")
    nc.sync.dma_start(out=g.sel_t[:], in_=g.sel_d[:])


def _front_stageA(g, b):
    """x+pos -> xz -> (z slice, xh) -> pw1 -> h12 (full [MID, L])."""
    nc, fp, fps = g.nc, g.fp, g.fps
    for (c0, nf) in MM_CHUNKS:
        xin = [fp.tile([128, 512], BF16, tag=f"xin{t}", bufs=2, name=f"xin{t}") for t in range(2)]
        for t in range(2):
            nc.sync.dma_start(out=xin[t][:, :nf],
                              in_=g.xT_d[b, t * 128:(t + 1) * 128, c0:c0 + nf])
        xhc = [fp.tile([128, 512], BF16, tag=f"xhc{m}", bufs=2, name=f"xhc{m}") for m in range(4)]
        for m in range(4):
            ps = fps.tile([128, 512], F32, tag="fps", bufs=2, name="ps_xz")
            for kt in range(2):
                nc.tensor.matmul(ps[:, :nf],
                                 lhsT=g.w_in_t[kt][:, m * 128:(m + 1) * 128],
                                 rhs=xin[kt][:, :nf],
                                 start=(kt == 0), stop=(kt == 1))
            nc.scalar.activation(out=xhc[m][:, :nf], in_=ps[:, :nf], func=AF.Copy)
        psz = fps.tile([ESH, 512], F32, tag="fps", bufs=2, name="ps_z")
        for kt in range(2):
            nc.tensor.matmul(psz[:, :nf],
                             lhsT=g.w_in_t[kt][:, E:E + ESH],
                             rhs=xin[kt][:, :nf],
                             start=(kt == 0), stop=(kt == 1))
        nc.scalar.activation(out=g.z_sl[b][:, c0:c0 + nf], in_=psz[:, :nf],
                             func=AF.Copy)
        ps1 = fps.tile([MID, 512], F32, tag="fps", bufs=2, name="ps_pw1")
        for kt in range(4):
            nc.tensor.matmul(ps1[:, :nf], lhsT=g.w_pw1_t[kt][:],
                             rhs=xhc[kt][:, :nf],
                             start=(kt == 0), stop=(kt == 3))
        nc.scalar.activation(out=g.h12[:, c0:c0 + nf], in_=ps1[:, :nf],
                             func=AF.Identity, bias=g.pw1b_t[:])


def _front_conv(g, b):
    """Depthwise 3x3 on h12 -> accB (bf16 copy for pw2 rhs)."""
    nc, fp = g.nc, g.fp
    acc = fp.tile([MID, L], F32, tag="dwacc")
    acc3 = acc[:].rearrange("p (h w) -> p h w", w=W)
    h3 = g.h12[:].rearrange("p (h w) -> p h w", w=W)
    nc.vector.tensor_scalar(out=acc[:], in0=g.h12[:],
                            scalar1=g.dwtap_t[:, 4:5], scalar2=None,
                            op0=OP.mult)
    for ky in range(3):
        for kx in range(3):
            if ky == 1 and kx == 1:
                continue
            dy, dx = ky - 1, kx - 1
            h0, h1 = max(0, -dy), H - max(0, dy)
            w0, w1 = max(0, -dx), W - max(0, dx)
            nc.vector.scalar_tensor_tensor(
                out=acc3[:, h0:h1, w0:w1],
                in0=h3[:, h0 + dy:h1 + dy, w0 + dx:w1 + dx],
                scalar=g.dwtap_t[:, ky * 3 + kx:ky * 3 + kx + 1],
                in1=acc3[:, h0:h1, w0:w1],
                op0=OP.mult, op1=OP.add)
    nc.scalar.activation(out=g.accB[:], in_=acc[:], func=AF.Copy)


def _front_stageB(g, b):
    """pw2+SiLU -> xc chunks -> (xc_sl slice, x_dbl, delta, B/C to DRAM)."""
    nc, fp, fps = g.nc, g.fp, g.fps
    for (c0, nf) in MM_CHUNKS:
        xcc = [fp.tile([128, 512], BF16, tag=f"xcc{m}", bufs=2, name=f"xcc{m}") for m in range(4)]
        for m in range(4):
            ps2 = fps.tile([128, 512], F32, tag="fps", bufs=2, name="ps_pw2")
            nc.tensor.matmul(ps2[:, :nf],
                             lhsT=g.w_pw2_t[:, m * 128:(m + 1) * 128],
                             rhs=g.accB[:, c0:c0 + nf], start=True, stop=True)
            nc.scalar.activation(out=xcc[m][:, :nf], in_=ps2[:, :nf],
                                 func=AF.Silu)
        nc.sync.dma_start(out=g.xc_sl[b][:, c0:c0 + nf], in_=xcc[0][0:ESH, :nf])
        ps3 = fps.tile([64, 512], F32, tag="fps", bufs=2, name="ps_xdbl")
        for kt in range(4):
            nc.tensor.matmul(ps3[:, :nf], lhsT=g.w_xp_t[kt][:],
                             rhs=xcc[kt][:, :nf],
                             start=(kt == 0), stop=(kt == 3))
        nc.scalar.activation(out=g.xdbl[:, c0:c0 + nf], in_=ps3[:, :nf],
                             func=AF.Copy)
        ps4 = fps.tile([ESH, 512], F32, tag="fps", bufs=2, name="ps_dt")
        nc.tensor.matmul(ps4[:, :nf], lhsT=g.w_dt_t[:],
                         rhs=g.xdbl[0:R, c0:c0 + nf], start=True, stop=True)
        nc.scalar.activation(out=g.dtlin[:, c0:c0 + nf], in_=ps4[:, :nf],
                             func=AF.Copy)
    # softplus(v) = ln(1 + exp(v)); |v| < ~10 here so exp cannot overflow.
    # Batched over full L to avoid ACT table reloads inside the chunk loop.
    nc.scalar.activation(out=g.delta_rep[b][0:ESH, :], in_=g.dtlin[:],
                         func=AF.Exp, bias=g.spb_t[:])
    nc.scalar.activation(out=g.delta_rep[b][0:ESH, :],
                         in_=g.delta_rep[b][0:ESH, :], func=AF.Ln, bias=1.0)
    # B/C rows to DRAM for later partition-broadcast loads
    nc.sync.dma_start(out=g.bsrc[b][:], in_=g.xdbl[R:R + N, :])
    nc.sync.dma_start(out=g.csrc[b][:], in_=g.xdbl[R + N:R + 2 * N, :])
    # duplicate delta rows [0:64] -> [64:128]
    nc.sync.dma_start(out=g.delta_rep[b][ESH:2 * ESH, :],
                      in_=g.delta_rep[b][0:ESH, :])


def _scan_prep(g, b):
    """y_acc init (D*u skip), dA exps, per-direction du tiles."""
    nc, sp, wp = g.nc, g.sp, g.wp
    nc.scalar.activation(out=g.y_acc[b][:], in_=g.xc_sl[b][:],
                         func=AF.Identity, bias=g.dpb_t[:], scale=g.dp4_t[:])
    for j in range(NJ):
        g.dA[j] = sp.tile([128, L], BF16, tag=f"dA{j}", name=f"dA{j}")
        nc.scalar.activation(out=g.dA[j][:], in_=g.delta_rep[b][:],
                             func=AF.Exp, scale=g.ascale_t[:, j:j + 1])
    xc3 = g.xc_sl[b][:].rearrange("p (h w) -> p h w", w=W)
    xcT = g.xc_sl[b][:].rearrange("p (h w) -> p w h", w=W)
    for k in range(4):
        src = [xc3, xc3[:, ::-1, ::-1], xcT, xcT[:, ::-1, ::-1]][k]
        u_tmp = wp.tile([ESH, L], BF16, tag="u_tmp", bufs=1)
        u3 = u_tmp[:].rearrange("p (a c) -> p a c", c=W)
        nc.scalar.activation(out=u3, in_=src, func=AF.Identity,
                             bias=g.dire_t[:, k:k + 1])
        g.du[k] = sp.tile([128, L], BF16, tag=f"du{k}", name=f"du{k}")
        nc.vector.tensor_tensor(out=g.du[k][0:ESH, :],
                                in0=g.delta_rep[b][0:ESH, :],
                                in1=u_tmp[:], op=OP.mult)
        nc.sync.dma_start(out=g.du[k][ESH:2 * ESH, :], in_=g.du[k][0:ESH, :])


def _scan_k(g, b, k):
    """One direction's 8 scan tiles + y accumulation."""
    nc, wp, yps = g.nc, g.wp, g.yps
    ypsum = [yps.tile([ESH, HALF], F32, tag=f"yps{h}", name=f"yps{h}") for h in range(2)]
    for j in range(NJ):
        B_t = wp.tile([128, L], BF16, tag="B_t", bufs=2)
        C_t = wp.tile([128, L], BF16, tag="C_t", bufs=2)
        for ns in range(2):
            nc.sync.dma_start(
                out=B_t[ns * ESH:(ns + 1) * ESH, :],
                in_=g.bsrc[b][2 * j + ns:2 * j + ns + 1, :]
                .to_broadcast((ESH, L)))
            nc.sync.dma_start(
                out=C_t[ns * ESH:(ns + 1) * ESH, :],
                in_=g.csrc[b][2 * j + ns:2 * j + ns + 1, :]
                .to_broadcast((ESH, L)))
        dbu = wp.tile([128, L], BF16, tag="workA", bufs=2)
        nc.vector.tensor_tensor(out=dbu[:], in0=g.du[k][:], in1=B_t[:],
                                op=OP.mult)
        h_t = wp.tile([128, L], BF16, tag="workH", bufs=2)
        nc.vector.tensor_tensor_scan(out=h_t[:], data0=g.dA[j][:],
                                     data1=dbu[:], initial=0.0,
                                     op0=OP.mult, op1=OP.add)
        hc = wp.tile([128, L], BF16, tag="workA", bufs=2)
        nc.vector.tensor_tensor(out=hc[:], in0=h_t[:], in1=C_t[:], op=OP.mult)
        for hh in range(2):
            for (c0, nf) in MM_CHUNKS_HALF:
                nc.tensor.matmul(
                    ypsum[hh][:, c0:c0 + nf],
                    lhsT=g.sel_t[:],
                    rhs=hc[:, hh * HALF + c0:hh * HALF + c0 + nf],
                    start=(j == 0), stop=(j == NJ - 1))
    # accumulate un-permuted ys_k into y_acc
    for hh in range(2):
        pv = ypsum[hh][:]
        if k == 0:
            dst = g.y_acc[b][:, hh * HALF:(hh + 1) * HALF]
            srcv = pv
        elif k == 1:
            dst = g.y_acc[b][:, (1 - hh) * HALF:(2 - hh) * HALF]
            srcv = pv[:, ::-1]
        elif k == 2:
            # ys[i], i=a*48+b_ -> l = b_*48+a ; half hh: a in [24hh,24hh+24)
            dst = g.y_acc[b][:].rearrange("p (bb a) -> p bb a", a=W)[
                :, :, 24 * hh:24 * hh + 24]
            srcv = pv.rearrange("p (a bb) -> p bb a", bb=W)
        else:
            dst = g.y_acc[b][:].rearrange("p (bb a) -> p bb a", a=W)[
                :, :, 24 * (1 - hh):24 * (1 - hh) + 24]
            srcv = pv.rearrange("p (a bb) -> p bb a", bb=W)[:, ::-1, ::-1]
        nc.vector.tensor_tensor(out=dst, in0=srcv, in1=dst, op=OP.add)


def _finish_b(g, b):
    """y_fin = y_acc * silu(z); out_partial = W_out^T @ y_fin."""
    nc, wp, fps = g.nc, g.wp, g.fps
    sz = wp.tile([ESH, L], BF16, tag="u_tmp", bufs=1)
    nc.scalar.activation(out=sz[:], in_=g.z_sl[b][:], func=AF.Silu)
    yv = wp.tile([ESH, L], BF16, tag="yv", bufs=1)
    for (c0, nf) in MM_CHUNKS:
        nc.vector.tensor_tensor(out=yv[:, c0:c0 + nf],
                                in0=g.y_acc[b][:, c0:c0 + nf],
                                in1=sz[:, c0:c0 + nf], op=OP.mult)
    for m in range(2):
        for (c0, nf) in MM_CHUNKS:
            po = fps.tile([128, 512], F32, tag="fps", bufs=2, name="ps_out")
            nc.tensor.matmul(po[:, :nf],
                             lhsT=g.w_out_t[:, m * 128:(m + 1) * 128],
                             rhs=yv[:, c0:c0 + nf], start=True, stop=True)
            osb = wp.tile([128, 512], F32, tag="osb", bufs=1)
            nc.scalar.activation(out=osb[:, :nf], in_=po[:, :nf], func=AF.Copy)
            nc.sync.dma_start(out=g.out_d[b, m * 128:(m + 1) * 128, c0:c0 + nf],
                              in_=osb[:, :nf])


def _r32r(a):
    """Round fp32 -> fp32r (TF32-like, 10 explicit mantissa bits)."""
    b = np.ascontiguousarray(a, np.float32).view(np.uint32)
    return (((b.astype(np.uint64) + 0x1000) & 0xFFFFE000)
            .astype(np.uint32).view(np.float32))


def _bf16(a):
    return np.ascontiguousarray(np.asarray(a, np.float32)).astype(
        ml_dtypes.bfloat16)


def _host_prep(inputs):
    x = np.asarray(inputs["x"], np.float32)
    W_pos = np.asarray(inputs["W_pos"], np.float32)
    b_pos = np.asarray(inputs["b_pos"], np.float32)
    W_in = np.asarray(inputs["W_in"], np.float32)
    pw1_w = np.asarray(inputs["pw1_w"], np.float32)
    pw1_b = np.asarray(inputs["pw1_b"], np.float32)
    dw_w = np.asarray(inputs["dw_w"], np.float32)
    pw2_w = np.asarray(inputs["pw2_w"], np.float32)
    W_xproj = np.asarray(inputs["W_xproj"], np.float32)
    W_dt = np.asarray(inputs["W_dt"], np.float32)
    b_dt = np.asarray(inputs["b_dt"], np.float32)
    A_log = np.asarray(inputs["A_log"], np.float32)
    Dp = np.asarray(inputs["Dp"], np.float32)
    dir_emb = np.asarray(inputs["dir_emb"], np.float32)
    W_out = np.asarray(inputs["W_out"], np.float32)

    gy, gx = np.meshgrid(np.arange(H, dtype=np.float32),
                         np.arange(W, dtype=np.float32), indexing="ij")
    coords = np.stack([gy, gx], -1) / (H - 1) * 2 - 1
    pos = (coords.reshape(L, 2) @ W_pos + b_pos).astype(np.float32)

    common = {
        "xT": _bf16((x + pos[None]).transpose(0, 2, 1)),
        "w_pw1": _bf16(pw1_w.reshape(MID, E).T),
        "pw1b": np.ascontiguousarray(pw1_b.reshape(MID, 1)),
        "dwtap": np.ascontiguousarray(dw_w.reshape(MID, 9)),
    }
    w_pw2_base = pw2_w.reshape(E, MID).T  # (MID, E)
    A = -np.exp(A_log)  # (E, N)

    sel = np.zeros((2 * ESH, ESH), np.float32)
    for p in range(2 * ESH):
        sel[p, p % ESH] = 1.0
    sel = sel.astype(ml_dtypes.bfloat16)

    in_maps = []
    for c in range(NCORES):
        e0 = c * ESH
        sl = slice(e0, e0 + ESH)
        A_sl = A[sl]  # (64, 16)
        ascale = np.empty((2 * ESH, NJ), np.float32)
        for p in range(2 * ESH):
            for j in range(NJ):
                ascale[p, j] = A_sl[p % ESH, 2 * j + p // ESH]
        m = dict(common)
        # channel permutation putting this core's slice at rows [0:64]
        perm = np.concatenate([np.arange(e0, e0 + ESH),
                               np.arange(0, e0),
                               np.arange(e0 + ESH, E)])
        m["w_pw2"] = _bf16(w_pw2_base[:, perm])
        m["w_xp"] = _bf16(np.concatenate(
            [W_xproj[perm, :], np.zeros((E, 64 - (R + 2 * N)), np.float32)],
            axis=1))
        m["w_in"] = _bf16(
            np.concatenate([W_in[:, :E], W_in[:, E + e0:E + e0 + ESH]], axis=1))
        m["w_dt"] = _bf16(W_dt[:, sl])
        m["spb"] = np.ascontiguousarray((2.0 * b_dt[sl]).reshape(ESH, 1))
        m["ascale"] = ascale
        m["dire"] = np.ascontiguousarray(dir_emb[:, sl].T)
        m["dp4"] = np.ascontiguousarray((4.0 * Dp[sl]).reshape(ESH, 1))
        m["dpb"] = np.ascontiguousarray(
            (Dp[sl] * dir_emb[:, sl].sum(0)).reshape(ESH, 1))
        m["w_out"] = _bf16(W_out[sl, :])
        m["sel"] = sel
        in_maps.append(m)
    return in_maps


_PROGRAM = None
_LAST_RESULTS = None
_LAST_INSTS = None


def _get_program():
    global _PROGRAM
    if _PROGRAM is None:
        _PROGRAM = build_program()
    return _PROGRAM


def kernel(**inputs):
    global _LAST_EXEC_NS, _LAST_RESULTS
    assert int(inputs["H"]) == H and int(inputs["W"]) == W
    in_maps = _host_prep(inputs)
    if TRACE:
        _install_profile_shim()
    res = run_bass_kernel_spmd(_get_program(), in_maps,
                               list(range(NCORES)), trace=TRACE)
    _LAST_EXEC_NS = res.exec_time_ns
    _LAST_RESULTS = res.results
    global _LAST_INSTS
    _LAST_INSTS = res.instructions_and_trace
    out = np.zeros((B, DM, L), np.float32)
    for r in res.results:
        out += np.asarray(r["out"], np.float32)
    return np.ascontiguousarray(out.transpose(0, 2, 1))


# revision 23
# speedup vs baseline: 1.3204x; 1.0433x over previous
"""Trainium2 Bass kernel for the LIDAR2D 4-direction selective-scan block.

Sharding: (batch, d_inner/4). Core c handles batch c//4 and a 128-channel
slice of d_inner (E=512): the host passes x[b] and channel-sliced scan
params per core, so the program stays SPMD-identical. Each core computes
the full-E front for its batch once, then scans its 128 channels as two
64-channel groups (cg) x 16 states x 4 directions, and emits a partial
out-projection (Dm, L). The host sums 4 partials per batch.

Scan layout per core: tiles of [128 partitions = (nsub in {0,1}) x (64
channels), free = L] — 8 tiles j=0..7 cover states n = 2j + nsub. The
recurrence h_t = exp(A*delta_t)*h_{t-1} + delta_t*B_t*u_t runs in a single
DVE tensor_tensor_scan per tile (2 cyc/elem — the hardware floor and the
kernel's critical path). y_t = C_t . h_t is a PE matmul with a 0/1
selection matrix contracting the two nsub rows per channel, accumulated
over j in PSUM. Direction permutations (reverse / spatial transpose) are
pure access-pattern tricks on reads/writes.

Perf structure: the front runs in bf16 (1 PE pass/col) with per-512-chunk
transient tiles. With one batch per core there is no second front to
overlap; the DVE stream is the 64 scans plus their elementwise feeds, and
the PE/Act fronts run only in the ~100us head.
"""

import os
import sys

for _p in ("/opt/trn_rl_repo", os.path.expanduser("~/.axon_site/_ro/trn_rl_repo")):
    if os.path.isdir(_p) and _p not in sys.path:
        sys.path.insert(0, _p)

import numpy as np
import ml_dtypes

import concourse.bass as bass
import concourse.bacc as bacc
import concourse.mybir as mybir
from concourse.tile import TileContext
from concourse.bass_utils import run_bass_kernel_spmd

F32 = mybir.dt.float32
F32R = mybir.dt.float32r
BF16 = mybir.dt.bfloat16
AF = mybir.ActivationFunctionType
OP = mybir.AluOpType

# Problem shape (hardcoded per the harness contract).
B, L, DM, E, N, R, MID, H, W = 2, 2304, 256, 512, 16, 16, 32, 48, 48
NCORES = 8
ESH = E // NCORES          # 64 channels per core
NJ = N // 2                # 8 scan tiles per (b, k); rows = (nsub, e_loc)
HALF = L // 2              # 1152, for PSUM-sized y accumulation

TRACE = bool(os.environ.get("KERNEL_TRACE"))
_LAST_EXEC_NS = None


def _install_profile_shim():
    """Make run_bass_kernel_spmd(trace=True) work in this container:
    register the NTFF hook (antenv.axon_hooks is absent here) and stub
    the S3 artifact upload."""
    import types
    try:
        from antenv.axon_hooks import get_axon_ntff_profile_hook  # noqa: F401
    except ImportError:
        import antenv
        mod = types.ModuleType("antenv.axon_hooks")
        mod._HOOK = None
        mod.set_axon_ntff_profile_hook = lambda h: setattr(mod, "_HOOK", h)
        mod.get_axon_ntff_profile_hook = lambda: mod._HOOK
        sys.modules["antenv.axon_hooks"] = mod
        antenv.axon_hooks = mod
        try:
            from trn_agent_boot.trn_boot import _ntff_profile_via_ctypes
            hook = _ntff_profile_via_ctypes("/opt/axon/libaxon_pjrt.so")
            if hook is not None:
                mod._HOOK = hook
        except Exception as e:  # pragma: no cover
            print(f"profile shim: hook install failed: {e}")
    import concourse.bass_utils as bu
    bu.upload_artifacts = lambda tmpdir: f"file://{tmpdir}"


def _chunks(total, step):
    out = []
    c0 = 0
    while c0 < total:
        out.append((c0, min(step, total - c0)))
        c0 += step
    return out


MM_CHUNKS = _chunks(L, 512)          # matmul free-dim chunks over full L
MM_CHUNKS_HALF = _chunks(HALF, 512)  # chunks within a 1152 half


class Env:
    pass


def build_program():
    nc = bacc.Bacc()
    g = Env()
    g.nc = nc

    # ---- DRAM parameters (same shapes on every core; values differ) ----
    g.xT_d = nc.declare_dram_parameter("xT", [DM, L], BF16, isOutput=False)
    g.w_in_d = nc.declare_dram_parameter("w_in", [DM, E + 2 * ESH], BF16, isOutput=False)
    g.w_pw1_d = nc.declare_dram_parameter("w_pw1", [E, MID], BF16, isOutput=False)
    g.pw1b_d = nc.declare_dram_parameter("pw1b", [MID, 1], F32, isOutput=False)
    g.dwtap_d = nc.declare_dram_parameter("dwtap", [MID, 9], F32, isOutput=False)
    g.w_pw2_d = nc.declare_dram_parameter("w_pw2", [MID, E], BF16, isOutput=False)
    g.w_xp_d = nc.declare_dram_parameter("w_xp", [E, 64], BF16, isOutput=False)
    g.w_dt_d = nc.declare_dram_parameter("w_dt", [R, 2 * ESH], BF16, isOutput=False)
    g.spb_d = nc.declare_dram_parameter("spb", [2, ESH, 1], F32, isOutput=False)
    g.ascale_d = nc.declare_dram_parameter("ascale", [2, 2 * ESH, NJ], F32, isOutput=False)
    g.dire_d = nc.declare_dram_parameter("dire", [2, ESH, 4], F32, isOutput=False)
    g.dp4_d = nc.declare_dram_parameter("dp4", [2, ESH, 1], F32, isOutput=False)
    g.dpb_d = nc.declare_dram_parameter("dpb", [2, ESH, 1], F32, isOutput=False)
    g.w_out_d = nc.declare_dram_parameter("w_out", [2, ESH, DM], BF16, isOutput=False)
    g.sel_d = nc.declare_dram_parameter("sel", [2 * ESH, ESH], BF16, isOutput=False)
    g.out_d = nc.declare_dram_parameter("out", [DM, L], F32, isOutput=True)

    with TileContext(nc) as tc:
        g.tc = tc
        with tc.tile_pool(name="const", bufs=1) as cp, \
             tc.tile_pool(name="persist", bufs=1) as pp, \
             tc.tile_pool(name="front", bufs=1) as fp, \
             tc.tile_pool(name="scan", bufs=1) as sp, \
             tc.tile_pool(name="work", bufs=2) as wp, \
             tc.tile_pool(name="fpsum", bufs=2, space="PSUM") as fps, \
             tc.tile_pool(name="ypsum", bufs=1, space="PSUM") as yps, \
             tc.tile_pool(name="bounce", bufs=1, space="DRAM") as bp:
            g.cp, g.pp, g.fp, g.sp, g.wp, g.fps, g.yps, g.bp = \
                cp, pp, fp, sp, wp, fps, yps, bp
            _load_consts(g)

            # persistent per-b products of the front
            g.xc_sl = [pp.tile([ESH, L], BF16, tag=f"xc_sl{b}", name=f"xc_sl{b}") for b in range(B)]
            g.delta_rep = [pp.tile([128, L], F32, tag=f"drep{b}", name=f"drep{b}") for b in range(B)]
            g.z_sl = [pp.tile([ESH, L], BF16, tag=f"z{b}", name=f"z{b}") for b in range(B)]
            g.y_acc = [pp.tile([ESH, L], F32, tag=f"yacc{b}", name=f"yacc{b}") for b in range(B)]
            g.bsrc = bp.tile([N, L], BF16, tag="bsrc", name="bsrc")
            g.csrc = bp.tile([N, L], BF16, tag="csrc", name="csrc")
            # h12/acc full-L per-b (conv needs the whole plane)
            g.h12 = fp.tile([MID, L], BF16, tag="h12", name="h12")
            g.accB = fp.tile([MID, L], BF16, tag="accB", name="accB")
            g.xdbl = fp.tile([64, L], BF16, tag="xdbl", name="xdbl")
            g.dtlin = [fp.tile([ESH, L], BF16, tag=f"dtlin{c}", name=f"dtlin{c}") for c in range(2)]
            g.dA = [None] * NJ
            g.yv = [None, None]

            # ---- emission schedule (one batch per core, two cg groups) ----
            _front_stageA(g)
            _front_conv(g)
            _front_stageB(g)
            _scan_prep(g, 0)
            for k in range(4):
                _scan_k(g, 0, k)
            _finish_yv(g, 0)
            _scan_prep(g, 1)
            for k in range(4):
                _scan_k(g, 1, k)
            _finish_yv(g, 1)
            _out_proj(g)

    nc.finalize()
    return nc


def _load_consts(g):
    nc, cp = g.nc, g.cp
    g.w_in_t = [cp.tile([128, E + 2 * ESH], BF16, tag=f"w_in{t}", name=f"w_in{t}") for t in range(2)]
    for t in range(2):
        nc.sync.dma_start(out=g.w_in_t[t][:], in_=g.w_in_d[t * 128:(t + 1) * 128, :])
    g.w_pw1_t = [cp.tile([128, MID], BF16, tag=f"w_pw1{t}", name=f"w_pw1{t}") for t in range(4)]
    for t in range(4):
        nc.sync.dma_start(out=g.w_pw1_t[t][:], in_=g.w_pw1_d[t * 128:(t + 1) * 128, :])
    g.pw1b_t = cp.tile([MID, 1], F32, tag="pw1b", name="pw1b")
    nc.sync.dma_start(out=g.pw1b_t[:], in_=g.pw1b_d[:])
    g.dwtap_t = cp.tile([MID, 9], F32, tag="dwtap", name="dwtap")
    nc.sync.dma_start(out=g.dwtap_t[:], in_=g.dwtap_d[:])
    g.w_pw2_t = cp.tile([MID, E], BF16, tag="w_pw2", name="w_pw2")
    nc.sync.dma_start(out=g.w_pw2_t[:], in_=g.w_pw2_d[:])
    g.w_xp_t = [cp.tile([128, 64], BF16, tag=f"w_xp{t}", name=f"w_xp{t}") for t in range(4)]
    for t in range(4):
        nc.sync.dma_start(out=g.w_xp_t[t][:], in_=g.w_xp_d[t * 128:(t + 1) * 128, :])
    g.w_dt_t = cp.tile([R, 2 * ESH], BF16, tag="w_dt", name="w_dt")
    nc.sync.dma_start(out=g.w_dt_t[:], in_=g.w_dt_d[:])
    g.spb_t = [cp.tile([ESH, 1], F32, tag=f"spb{c}", name=f"spb{c}") for c in range(2)]
    g.ascale_t = [cp.tile([2 * ESH, NJ], F32, tag=f"ascale{c}", name=f"ascale{c}") for c in range(2)]
    g.dire_t = [cp.tile([ESH, 4], F32, tag=f"dire{c}", name=f"dire{c}") for c in range(2)]
    g.dp4_t = [cp.tile([ESH, 1], F32, tag=f"dp4{c}", name=f"dp4{c}") for c in range(2)]
    g.dpb_t = [cp.tile([ESH, 1], F32, tag=f"dpb{c}", name=f"dpb{c}") for c in range(2)]
    g.w_out_t = [cp.tile([ESH, DM], BF16, tag=f"w_out{c}", name=f"w_out{c}") for c in range(2)]
    for c in range(2):
        nc.sync.dma_start(out=g.spb_t[c][:], in_=g.spb_d[c])
        nc.sync.dma_start(out=g.ascale_t[c][:], in_=g.ascale_d[c])
        nc.sync.dma_start(out=g.dire_t[c][:], in_=g.dire_d[c])
        nc.sync.dma_start(out=g.dp4_t[c][:], in_=g.dp4_d[c])
        nc.sync.dma_start(out=g.dpb_t[c][:], in_=g.dpb_d[c])
        nc.sync.dma_start(out=g.w_out_t[c][:], in_=g.w_out_d[c])
    g.sel_t = cp.tile([2 * ESH, ESH], BF16, tag="sel", name="sel")
    nc.sync.dma_start(out=g.sel_t[:], in_=g.sel_d[:])


def _front_stageA(g):
    """x+pos -> xz -> (z slices, xh) -> pw1 -> h12 (full [MID, L])."""
    nc, fp, fps = g.nc, g.fp, g.fps
    for (c0, nf) in MM_CHUNKS:
        xin = [fp.tile([128, 512], BF16, tag=f"xin{t}", bufs=2, name=f"xin{t}") for t in range(2)]
        for t in range(2):
            nc.sync.dma_start(out=xin[t][:, :nf],
                              in_=g.xT_d[t * 128:(t + 1) * 128, c0:c0 + nf])
        xhc = [fp.tile([128, 512], BF16, tag=f"xhc{m}", bufs=2, name=f"xhc{m}") for m in range(4)]
        for m in range(4):
            ps = fps.tile([128, 512], F32, tag="fps", bufs=2, name="ps_xz")
            for kt in range(2):
                nc.tensor.matmul(ps[:, :nf],
                                 lhsT=g.w_in_t[kt][:, m * 128:(m + 1) * 128],
                                 rhs=xin[kt][:, :nf],
                                 start=(kt == 0), stop=(kt == 1))
            nc.scalar.activation(out=xhc[m][:, :nf], in_=ps[:, :nf], func=AF.Copy)
        for cg in range(2):
            psz = fps.tile([ESH, 512], F32, tag="fps", bufs=2, name="ps_z")
            for kt in range(2):
                nc.tensor.matmul(psz[:, :nf],
                                 lhsT=g.w_in_t[kt][:, E + cg * ESH:E + (cg + 1) * ESH],
                                 rhs=xin[kt][:, :nf],
                                 start=(kt == 0), stop=(kt == 1))
            nc.scalar.activation(out=g.z_sl[cg][:, c0:c0 + nf], in_=psz[:, :nf],
                                 func=AF.Copy)
        ps1 = fps.tile([MID, 512], F32, tag="fps", bufs=2, name="ps_pw1")
        for kt in range(4):
            nc.tensor.matmul(ps1[:, :nf], lhsT=g.w_pw1_t[kt][:],
                             rhs=xhc[kt][:, :nf],
                             start=(kt == 0), stop=(kt == 3))
        nc.scalar.activation(out=g.h12[:, c0:c0 + nf], in_=ps1[:, :nf],
                             func=AF.Identity, bias=g.pw1b_t[:])


def _front_conv(g):
    """Depthwise 3x3 on h12 -> accB (bf16 copy for pw2 rhs)."""
    nc, fp = g.nc, g.fp
    acc = fp.tile([MID, L], BF16, tag="dwacc")
    acc3 = acc[:].rearrange("p (h w) -> p h w", w=W)
    h3 = g.h12[:].rearrange("p (h w) -> p h w", w=W)
    nc.vector.tensor_scalar(out=acc[:], in0=g.h12[:],
                            scalar1=g.dwtap_t[:, 4:5], scalar2=None,
                            op0=OP.mult)
    for ky in range(3):
        for kx in range(3):
            if ky == 1 and kx == 1:
                continue
            dy, dx = ky - 1, kx - 1
            h0, h1 = max(0, -dy), H - max(0, dy)
            w0, w1 = max(0, -dx), W - max(0, dx)
            nc.vector.scalar_tensor_tensor(
                out=acc3[:, h0:h1, w0:w1],
                in0=h3[:, h0 + dy:h1 + dy, w0 + dx:w1 + dx],
                scalar=g.dwtap_t[:, ky * 3 + kx:ky * 3 + kx + 1],
                in1=acc3[:, h0:h1, w0:w1],
                op0=OP.mult, op1=OP.add)
    nc.scalar.activation(out=g.accB[:], in_=acc[:], func=AF.Copy)


def _front_stageB(g):
    """pw2+SiLU -> xc chunks -> (xc_sl slice, x_dbl, delta, B/C to DRAM)."""
    nc, fp, fps = g.nc, g.fp, g.fps
    for (c0, nf) in MM_CHUNKS:
        xcc = [fp.tile([128, 512], BF16, tag=f"xcc{m}", bufs=2, name=f"xcc{m}") for m in range(4)]
        for m in range(4):
            ps2 = fps.tile([128, 512], F32, tag="fps", bufs=2, name="ps_pw2")
            nc.tensor.matmul(ps2[:, :nf],
                             lhsT=g.w_pw2_t[:, m * 128:(m + 1) * 128],
                             rhs=g.accB[:, c0:c0 + nf], start=True, stop=True)
            nc.scalar.activation(out=xcc[m][:, :nf], in_=ps2[:, :nf],
                                 func=AF.Silu)
        for cg in range(2):
            nc.sync.dma_start(out=g.xc_sl[cg][:, c0:c0 + nf],
                              in_=xcc[0][cg * ESH:(cg + 1) * ESH, :nf])
        ps3 = fps.tile([64, 512], F32, tag="fps", bufs=2, name="ps_xdbl")
        for kt in range(4):
            nc.tensor.matmul(ps3[:, :nf], lhsT=g.w_xp_t[kt][:],
                             rhs=xcc[kt][:, :nf],
                             start=(kt == 0), stop=(kt == 3))
        nc.scalar.activation(out=g.xdbl[:, c0:c0 + nf], in_=ps3[:, :nf],
                             func=AF.Copy)
        for cg in range(2):
            ps4 = fps.tile([ESH, 512], F32, tag="fps", bufs=2, name="ps_dt")
            nc.tensor.matmul(ps4[:, :nf],
                             lhsT=g.w_dt_t[:, cg * ESH:(cg + 1) * ESH],
                             rhs=g.xdbl[0:R, c0:c0 + nf], start=True, stop=True)
            nc.scalar.activation(out=g.dtlin[cg][:, c0:c0 + nf], in_=ps4[:, :nf],
                                 func=AF.Copy)
    # softplus(v) = ln(1 + exp(v)); |v| < ~10 here so exp cannot overflow.
    # Batched over full L to avoid ACT table reloads inside the chunk loop.
    for cg in range(2):
        nc.scalar.activation(out=g.delta_rep[cg][0:ESH, :], in_=g.dtlin[cg][:],
                             func=AF.Exp, bias=g.spb_t[cg][:])
        nc.scalar.activation(out=g.delta_rep[cg][0:ESH, :],
                             in_=g.delta_rep[cg][0:ESH, :], func=AF.Ln, bias=1.0)
        # duplicate delta rows [0:64] -> [64:128]
        nc.sync.dma_start(out=g.delta_rep[cg][ESH:2 * ESH, :],
                          in_=g.delta_rep[cg][0:ESH, :])
    # B/C rows to DRAM for later partition-broadcast loads
    nc.sync.dma_start(out=g.bsrc[:], in_=g.xdbl[R:R + N, :])
    nc.sync.dma_start(out=g.csrc[:], in_=g.xdbl[R + N:R + 2 * N, :])


def _scan_prep(g, cg):
    """y_acc init (D*u skip), dA exps, per-direction du tiles."""
    nc, sp, wp = g.nc, g.sp, g.wp
    nc.scalar.activation(out=g.y_acc[cg][:], in_=g.xc_sl[cg][:],
                         func=AF.Identity, bias=g.dpb_t[cg][:],
                         scale=g.dp4_t[cg][:])
    for j in range(NJ):
        g.dA[j] = sp.tile([128, L], BF16, tag=f"dA{j}", name=f"dA{j}")
        nc.scalar.activation(out=g.dA[j][:], in_=g.delta_rep[cg][:],
                             func=AF.Exp, scale=g.ascale_t[cg][:, j:j + 1])



def _scan_k(g, cg, k):
    """One direction's du + 8 scan tiles + y accumulation."""
    nc, wp, yps = g.nc, g.wp, g.yps
    xc3 = g.xc_sl[cg][:].rearrange("p (h w) -> p h w", w=W)
    xcT = g.xc_sl[cg][:].rearrange("p (h w) -> p w h", w=W)
    usrc = [xc3, xc3[:, ::-1, ::-1], xcT, xcT[:, ::-1, ::-1]][k]
    u_tmp = wp.tile([ESH, L], BF16, tag="u_tmp", bufs=1)
    u3 = u_tmp[:].rearrange("p (a c) -> p a c", c=W)
    nc.scalar.activation(out=u3, in_=usrc, func=AF.Identity,
                         bias=g.dire_t[cg][:, k:k + 1])
    du = wp.tile([128, L], BF16, tag="du", bufs=2)
    nc.vector.tensor_tensor(out=du[0:ESH, :], in0=g.delta_rep[cg][0:ESH, :],
                            in1=u_tmp[:], op=OP.mult)
    nc.sync.dma_start(out=du[ESH:2 * ESH, :], in_=du[0:ESH, :])
    ypsum = [yps.tile([ESH, HALF], F32, tag=f"yps{h}", name=f"yps{h}") for h in range(2)]
    for j in range(NJ):
        B_t = wp.tile([128, L], BF16, tag="B_t", bufs=2)
        C_t = wp.tile([128, L], BF16, tag="C_t", bufs=2)
        for ns in range(2):
            nc.sync.dma_start(
                out=B_t[ns * ESH:(ns + 1) * ESH, :],
                in_=g.bsrc[2 * j + ns:2 * j + ns + 1, :]
                .to_broadcast((ESH, L)))
            nc.sync.dma_start(
                out=C_t[ns * ESH:(ns + 1) * ESH, :],
                in_=g.csrc[2 * j + ns:2 * j + ns + 1, :]
                .to_broadcast((ESH, L)))
        dbu = wp.tile([128, L], BF16, tag="workA", bufs=2)
        nc.vector.tensor_tensor(out=dbu[:], in0=du[:], in1=B_t[:],
                                op=OP.mult)
        h_t = wp.tile([128, L], BF16, tag="workH", bufs=1)
        nc.vector.tensor_tensor_scan(out=h_t[:], data0=g.dA[j][:],
                                     data1=dbu[:], initial=0.0,
                                     op0=OP.mult, op1=OP.add)
        hc = wp.tile([128, L], BF16, tag="workA", bufs=2)
        nc.vector.tensor_tensor(out=hc[:], in0=h_t[:], in1=C_t[:], op=OP.mult)
        for hh in range(2):
            for (c0, nf) in MM_CHUNKS_HALF:
                nc.tensor.matmul(
                    ypsum[hh][:, c0:c0 + nf],
                    lhsT=g.sel_t[:],
                    rhs=hc[:, hh * HALF + c0:hh * HALF + c0 + nf],
                    start=(j == 0), stop=(j == NJ - 1))
    # accumulate un-permuted ys_k into y_acc
    for hh in range(2):
        pv = ypsum[hh][:]
        if k == 0:
            dst = g.y_acc[cg][:, hh * HALF:(hh + 1) * HALF]
            srcv = pv
        elif k == 1:
            dst = g.y_acc[cg][:, (1 - hh) * HALF:(2 - hh) * HALF]
            srcv = pv[:, ::-1]
        elif k == 2:
            # ys[i], i=a*48+b_ -> l = b_*48+a ; half hh: a in [24hh,24hh+24)
            dst = g.y_acc[cg][:].rearrange("p (bb a) -> p bb a", a=W)[
                :, :, 24 * hh:24 * hh + 24]
            srcv = pv.rearrange("p (a bb) -> p bb a", bb=W)
        else:
            dst = g.y_acc[cg][:].rearrange("p (bb a) -> p bb a", a=W)[
                :, :, 24 * (1 - hh):24 * (1 - hh) + 24]
            srcv = pv.rearrange("p (a bb) -> p bb a", bb=W)[:, ::-1, ::-1]
        nc.vector.tensor_tensor(out=dst, in0=srcv, in1=dst, op=OP.add)


def _finish_yv(g, cg):
    """yv[cg] = y_acc * silu(z) for this channel group."""
    nc, wp = g.nc, g.wp
    sz = wp.tile([ESH, L], BF16, tag="u_tmp", bufs=1)
    nc.scalar.activation(out=sz[:], in_=g.z_sl[cg][:], func=AF.Silu)
    g.yv[cg] = g.sp.tile([ESH, L], BF16, tag=f"yv{cg}", name=f"yv{cg}")
    nc.vector.tensor_tensor(out=g.yv[cg][:], in0=g.y_acc[cg][:], in1=sz[:],
                            op=OP.mult)


def _out_proj(g):
    """out_partial = sum_cg W_out[cg]^T @ yv[cg] (PSUM-accumulated)."""
    nc, wp, fps = g.nc, g.wp, g.fps
    for m in range(2):
        for (c0, nf) in MM_CHUNKS:
            po = fps.tile([128, 512], F32, tag="fps", bufs=2, name="ps_out")
            for cg in range(2):
                nc.tensor.matmul(po[:, :nf],
                                 lhsT=g.w_out_t[cg][:, m * 128:(m + 1) * 128],
                                 rhs=g.yv[cg][:, c0:c0 + nf],
                                 start=(cg == 0), stop=(cg == 1))
            osb = wp.tile([128, 512], F32, tag="osb", bufs=1)
            nc.scalar.activation(out=osb[:, :nf], in_=po[:, :nf], func=AF.Copy)
            nc.sync.dma_start(out=g.out_d[m * 128:(m + 1) * 128, c0:c0 + nf],
                              in_=osb[:, :nf])


def _r32r(a):
    """Round fp32 -> fp32r (TF32-like, 10 explicit mantissa bits)."""
    b = np.ascontiguousarray(a, np.float32).view(np.uint32)
    return (((b.astype(np.uint64) + 0x1000) & 0xFFFFE000)
            .astype(np.uint32).view(np.float32))


def _bf16(a):
    return np.ascontiguousarray(np.asarray(a, np.float32)).astype(
        ml_dtypes.bfloat16)


def _host_prep(inputs):
    x = np.asarray(inputs["x"], np.float32)
    W_pos = np.asarray(inputs["W_pos"], np.float32)
    b_pos = np.asarray(inputs["b_pos"], np.float32)
    W_in = np.asarray(inputs["W_in"], np.float32)
    pw1_w = np.asarray(inputs["pw1_w"], np.float32)
    pw1_b = np.asarray(inputs["pw1_b"], np.float32)
    dw_w = np.asarray(inputs["dw_w"], np.float32)
    pw2_w = np.asarray(inputs["pw2_w"], np.float32)
    W_xproj = np.asarray(inputs["W_xproj"], np.float32)
    W_dt = np.asarray(inputs["W_dt"], np.float32)
    b_dt = np.asarray(inputs["b_dt"], np.float32)
    A_log = np.asarray(inputs["A_log"], np.float32)
    Dp = np.asarray(inputs["Dp"], np.float32)
    dir_emb = np.asarray(inputs["dir_emb"], np.float32)
    W_out = np.asarray(inputs["W_out"], np.float32)

    gy, gx = np.meshgrid(np.arange(H, dtype=np.float32),
                         np.arange(W, dtype=np.float32), indexing="ij")
    coords = np.stack([gy, gx], -1) / (H - 1) * 2 - 1
    pos = (coords.reshape(L, 2) @ W_pos + b_pos).astype(np.float32)

    common = {
        "w_pw1": _bf16(pw1_w.reshape(MID, E).T),
        "pw1b": np.ascontiguousarray(pw1_b.reshape(MID, 1)),
        "dwtap": np.ascontiguousarray(dw_w.reshape(MID, 9)),
    }
    w_pw2_base = pw2_w.reshape(E, MID).T  # (MID, E)
    A = -np.exp(A_log)  # (E, N)
    xp = (x + pos[None]).transpose(0, 2, 1)  # (B, Dm, L)

    sel = np.zeros((2 * ESH, ESH), np.float32)
    for p in range(2 * ESH):
        sel[p, p % ESH] = 1.0
    sel = sel.astype(ml_dtypes.bfloat16)

    in_maps = []
    for c in range(NCORES):
        bcr = c // 4               # this core's batch
        e0 = (c % 4) * 2 * ESH     # this core's 128-channel slice
        sl = slice(e0, e0 + 2 * ESH)
        ascale = np.empty((2, 2 * ESH, NJ), np.float32)
        for cg in range(2):
            A_cg = A[e0 + cg * ESH:e0 + (cg + 1) * ESH]  # (64, 16)
            for p in range(2 * ESH):
                for j in range(NJ):
                    ascale[cg, p, j] = A_cg[p % ESH, 2 * j + p // ESH]
        m = dict(common)
        m["xT"] = _bf16(xp[bcr])
        # channel permutation putting this core's slice at rows [0:128]
        perm = np.concatenate([np.arange(e0, e0 + 2 * ESH),
                               np.arange(0, e0),
                               np.arange(e0 + 2 * ESH, E)])
        m["w_pw2"] = _bf16(w_pw2_base[:, perm])
        m["w_xp"] = _bf16(np.concatenate(
            [W_xproj[perm, :], np.zeros((E, 64 - (R + 2 * N)), np.float32)],
            axis=1))
        m["w_in"] = _bf16(
            np.concatenate([W_in[:, :E], W_in[:, E + e0:E + e0 + 2 * ESH]],
                           axis=1))
        m["w_dt"] = _bf16(W_dt[:, sl])
        m["spb"] = np.ascontiguousarray(
            (2.0 * b_dt[sl]).reshape(2, ESH, 1))
        m["ascale"] = ascale
        m["dire"] = np.ascontiguousarray(
            dir_emb[:, sl].T.reshape(2, ESH, 4))
        m["dp4"] = np.ascontiguousarray((4.0 * Dp[sl]).reshape(2, ESH, 1))
        m["dpb"] = np.ascontiguousarray(
            (Dp[sl] * dir_emb[:, sl].sum(0)).reshape(2, ESH, 1))
        m["w_out"] = _bf16(W_out[sl, :].reshape(2, ESH, DM))
        m["sel"] = sel
        in_maps.append(m)
    return in_maps


_PROGRAM = None
_LAST_RESULTS = None
_LAST_INSTS = None


def _get_program():
    global _PROGRAM
    if _PROGRAM is None:
        _PROGRAM = build_program()
    return _PROGRAM


def kernel(**inputs):
    global _LAST_EXEC_NS, _LAST_RESULTS
    assert int(inputs["H"]) == H and int(inputs["W"]) == W
    in_maps = _host_prep(inputs)
    if TRACE:
        _install_profile_shim()
    res = run_bass_kernel_spmd(_get_program(), in_maps,
                               list(range(NCORES)), trace=TRACE)
    _LAST_EXEC_NS = res.exec_time_ns
    _LAST_RESULTS = res.results
    global _LAST_INSTS
    _LAST_INSTS = res.instructions_and_trace
    out = np.zeros((B, DM, L), np.float32)
    for c, r in enumerate(res.results):
        out[c // 4] += np.asarray(r["out"], np.float32)
    return np.ascontiguousarray(out.transpose(0, 2, 1))


# revision 24
# speedup vs baseline: 1.3854x; 1.0492x over previous
"""Trainium2 Bass kernel for the LIDAR2D 4-direction selective-scan block.

Sharding: (batch, d_inner/4). Core c handles batch c//4 and a 128-channel
slice of d_inner (E=512): the host passes x[b] and channel-sliced scan
params per core, so the program stays SPMD-identical. Each core computes
the full-E front for its batch once, then scans its 128 channels as two
64-channel groups (cg) x 16 states x 4 directions, and emits a partial
out-projection (Dm, L). The host sums 4 partials per batch.

Scan layout per core: tiles of [128 partitions = (nsub in {0,1}) x (64
channels), free = L] — 8 tiles j=0..7 cover states n = 2j + nsub. The
recurrence h_t = exp(A*delta_t)*h_{t-1} + delta_t*B_t*u_t runs in a single
DVE tensor_tensor_scan per tile (2 cyc/elem — the hardware floor and the
kernel's critical path). y_t = C_t . h_t is a PE matmul with a 0/1
selection matrix contracting the two nsub rows per channel, accumulated
over j in PSUM. Direction permutations (reverse / spatial transpose) are
pure access-pattern tricks on reads/writes.

Perf structure: the front runs in bf16 (1 PE pass/col) with per-512-chunk
transient tiles. With one batch per core there is no second front to
overlap; the DVE stream is the 64 scans plus their elementwise feeds, and
the PE/Act fronts run only in the ~100us head.
"""

import os
import sys

for _p in ("/opt/trn_rl_repo", os.path.expanduser("~/.axon_site/_ro/trn_rl_repo")):
    if os.path.isdir(_p) and _p not in sys.path:
        sys.path.insert(0, _p)

import numpy as np
import ml_dtypes

import concourse.bass as bass
import concourse.bacc as bacc
import concourse.mybir as mybir
from concourse.tile import TileContext
from concourse.bass_utils import run_bass_kernel_spmd

F32 = mybir.dt.float32
F32R = mybir.dt.float32r
BF16 = mybir.dt.bfloat16
AF = mybir.ActivationFunctionType
OP = mybir.AluOpType

# Problem shape (hardcoded per the harness contract).
B, L, DM, E, N, R, MID, H, W = 2, 2304, 256, 512, 16, 16, 32, 48, 48
NCORES = 8
ESH = E // NCORES          # 64 channels per core
NJ = N // 2                # 8 scan tiles per (b, k); rows = (nsub, e_loc)
HALF = L // 2              # 1152, for PSUM-sized y accumulation

TRACE = bool(os.environ.get("KERNEL_TRACE"))
_LAST_EXEC_NS = None


def _install_profile_shim():
    """Make run_bass_kernel_spmd(trace=True) work in this container:
    register the NTFF hook (antenv.axon_hooks is absent here) and stub
    the S3 artifact upload."""
    import types
    try:
        from antenv.axon_hooks import get_axon_ntff_profile_hook  # noqa: F401
    except ImportError:
        import antenv
        mod = types.ModuleType("antenv.axon_hooks")
        mod._HOOK = None
        mod.set_axon_ntff_profile_hook = lambda h: setattr(mod, "_HOOK", h)
        mod.get_axon_ntff_profile_hook = lambda: mod._HOOK
        sys.modules["antenv.axon_hooks"] = mod
        antenv.axon_hooks = mod
        try:
            from trn_agent_boot.trn_boot import _ntff_profile_via_ctypes
            hook = _ntff_profile_via_ctypes("/opt/axon/libaxon_pjrt.so")
            if hook is not None:
                mod._HOOK = hook
        except Exception as e:  # pragma: no cover
            print(f"profile shim: hook install failed: {e}")
    import concourse.bass_utils as bu
    bu.upload_artifacts = lambda tmpdir: f"file://{tmpdir}"


def _chunks(total, step):
    out = []
    c0 = 0
    while c0 < total:
        out.append((c0, min(step, total - c0)))
        c0 += step
    return out


MM_CHUNKS = _chunks(L, 512)          # matmul free-dim chunks over full L
MM_CHUNKS_HALF = _chunks(HALF, 512)  # chunks within a 1152 half


class Env:
    pass


def build_program():
    nc = bacc.Bacc()
    g = Env()
    g.nc = nc

    # ---- DRAM parameters (same shapes on every core; values differ) ----
    g.xT_d = nc.declare_dram_parameter("xT", [DM, L], BF16, isOutput=False)
    g.w_in_d = nc.declare_dram_parameter("w_in", [DM, E + 2 * ESH], BF16, isOutput=False)
    g.w_pw1_d = nc.declare_dram_parameter("w_pw1", [E, MID], BF16, isOutput=False)
    g.pw1b_d = nc.declare_dram_parameter("pw1b", [MID, 1], F32, isOutput=False)
    g.dwtap_d = nc.declare_dram_parameter("dwtap", [MID, 9], F32, isOutput=False)
    g.w_pw2_d = nc.declare_dram_parameter("w_pw2", [MID, E], BF16, isOutput=False)
    g.w_xp_d = nc.declare_dram_parameter("w_xp", [E, 64], BF16, isOutput=False)
    g.w_dt_d = nc.declare_dram_parameter("w_dt", [R, 2 * ESH], BF16, isOutput=False)
    g.spb_d = nc.declare_dram_parameter("spb", [2, ESH, 1], F32, isOutput=False)
    g.ascale_d = nc.declare_dram_parameter("ascale", [2, 2 * ESH, NJ], F32, isOutput=False)
    g.dire_d = nc.declare_dram_parameter("dire", [2, ESH, 4], F32, isOutput=False)
    g.dp4_d = nc.declare_dram_parameter("dp4", [2, ESH, 1], F32, isOutput=False)
    g.dpb_d = nc.declare_dram_parameter("dpb", [2, ESH, 1], F32, isOutput=False)
    g.w_out_d = nc.declare_dram_parameter("w_out", [2, ESH, DM], BF16, isOutput=False)
    g.sel_d = nc.declare_dram_parameter("sel", [2 * ESH, ESH], BF16, isOutput=False)
    g.out_d = nc.declare_dram_parameter("out", [DM, L], F32, isOutput=True)

    with TileContext(nc) as tc:
        g.tc = tc
        with tc.tile_pool(name="const", bufs=1) as cp, \
             tc.tile_pool(name="persist", bufs=1) as pp, \
             tc.tile_pool(name="front", bufs=1) as fp, \
             tc.tile_pool(name="scan", bufs=1) as sp, \
             tc.tile_pool(name="work", bufs=2) as wp, \
             tc.tile_pool(name="fpsum", bufs=2, space="PSUM") as fps, \
             tc.tile_pool(name="ypsum", bufs=1, space="PSUM") as yps, \
             tc.tile_pool(name="bounce", bufs=1, space="DRAM") as bp:
            g.cp, g.pp, g.fp, g.sp, g.wp, g.fps, g.yps, g.bp = \
                cp, pp, fp, sp, wp, fps, yps, bp
            _load_consts(g)

            # persistent per-b products of the front
            g.xc_sl = [pp.tile([ESH, L], BF16, tag=f"xc_sl{b}", name=f"xc_sl{b}") for b in range(B)]
            g.delta_rep = [pp.tile([128, L], F32, tag=f"drep{b}", name=f"drep{b}") for b in range(B)]
            g.z_sl = [pp.tile([ESH, L], BF16, tag=f"z{b}", name=f"z{b}") for b in range(B)]
            g.y_acc = [pp.tile([ESH, L], F32, tag=f"yacc{b}", name=f"yacc{b}") for b in range(B)]
            g.bsrc = bp.tile([N, L], BF16, tag="bsrc", name="bsrc")
            g.csrc = bp.tile([N, L], BF16, tag="csrc", name="csrc")
            # h12/acc full-L per-b (conv needs the whole plane)
            g.h12 = fp.tile([MID, L], BF16, tag="h12", name="h12")
            g.accB = fp.tile([MID, L], BF16, tag="accB", name="accB")
            g.xdbl = fp.tile([64, L], BF16, tag="xdbl", name="xdbl")
            g.dtlin = [fp.tile([ESH, L], BF16, tag=f"dtlin{c}", name=f"dtlin{c}") for c in range(2)]
            g.dA = [None] * NJ
            g.yv = [None, None]

            # ---- emission schedule (one batch per core, two cg groups) ----
            _front_stageA(g)
            _front_conv(g)
            _front_stageB(g)
            _scan_prep(g, 0)
            for k in range(4):
                _scan_k(g, 0, k)
            _finish_yv(g, 0)
            _scan_prep(g, 1)
            for k in range(4):
                _scan_k(g, 1, k)
            _finish_yv(g, 1)
            _out_proj(g)

    nc.finalize()
    return nc


def _load_consts(g):
    nc, cp = g.nc, g.cp
    g.w_in_t = [cp.tile([128, E + 2 * ESH], BF16, tag=f"w_in{t}", name=f"w_in{t}") for t in range(2)]
    for t in range(2):
        nc.sync.dma_start(out=g.w_in_t[t][:], in_=g.w_in_d[t * 128:(t + 1) * 128, :])
    g.w_pw1_t = [cp.tile([128, MID], BF16, tag=f"w_pw1{t}", name=f"w_pw1{t}") for t in range(4)]
    for t in range(4):
        nc.sync.dma_start(out=g.w_pw1_t[t][:], in_=g.w_pw1_d[t * 128:(t + 1) * 128, :])
    g.pw1b_t = cp.tile([MID, 1], F32, tag="pw1b", name="pw1b")
    nc.sync.dma_start(out=g.pw1b_t[:], in_=g.pw1b_d[:])
    g.dwtap_t = cp.tile([MID, 9], F32, tag="dwtap", name="dwtap")
    nc.sync.dma_start(out=g.dwtap_t[:], in_=g.dwtap_d[:])
    g.w_pw2_t = cp.tile([MID, E], BF16, tag="w_pw2", name="w_pw2")
    nc.sync.dma_start(out=g.w_pw2_t[:], in_=g.w_pw2_d[:])
    g.w_xp_t = [cp.tile([128, 64], BF16, tag=f"w_xp{t}", name=f"w_xp{t}") for t in range(4)]
    for t in range(4):
        nc.sync.dma_start(out=g.w_xp_t[t][:], in_=g.w_xp_d[t * 128:(t + 1) * 128, :])
    g.w_dt_t = cp.tile([R, 2 * ESH], BF16, tag="w_dt", name="w_dt")
    nc.sync.dma_start(out=g.w_dt_t[:], in_=g.w_dt_d[:])
    g.spb_t = [cp.tile([ESH, 1], F32, tag=f"spb{c}", name=f"spb{c}") for c in range(2)]
    g.ascale_t = [cp.tile([2 * ESH, NJ], F32, tag=f"ascale{c}", name=f"ascale{c}") for c in range(2)]
    g.dire_t = [cp.tile([ESH, 4], F32, tag=f"dire{c}", name=f"dire{c}") for c in range(2)]
    g.dp4_t = [cp.tile([ESH, 1], F32, tag=f"dp4{c}", name=f"dp4{c}") for c in range(2)]
    g.dpb_t = [cp.tile([ESH, 1], F32, tag=f"dpb{c}", name=f"dpb{c}") for c in range(2)]
    g.w_out_t = [cp.tile([ESH, DM], BF16, tag=f"w_out{c}", name=f"w_out{c}") for c in range(2)]
    for c in range(2):
        nc.sync.dma_start(out=g.spb_t[c][:], in_=g.spb_d[c])
        nc.sync.dma_start(out=g.ascale_t[c][:], in_=g.ascale_d[c])
        nc.sync.dma_start(out=g.dire_t[c][:], in_=g.dire_d[c])
        nc.sync.dma_start(out=g.dp4_t[c][:], in_=g.dp4_d[c])
        nc.sync.dma_start(out=g.dpb_t[c][:], in_=g.dpb_d[c])
        nc.sync.dma_start(out=g.w_out_t[c][:], in_=g.w_out_d[c])
    g.sel_t = cp.tile([2 * ESH, ESH], BF16, tag="sel", name="sel")
    nc.sync.dma_start(out=g.sel_t[:], in_=g.sel_d[:])


def _front_stageA(g):
    """x+pos -> xz -> (z slices, xh) -> pw1 -> h12 (full [MID, L])."""
    nc, fp, fps = g.nc, g.fp, g.fps
    for (c0, nf) in MM_CHUNKS:
        xin = [fp.tile([128, 512], BF16, tag=f"xin{t}", bufs=2, name=f"xin{t}") for t in range(2)]
        for t in range(2):
            nc.sync.dma_start(out=xin[t][:, :nf],
                              in_=g.xT_d[t * 128:(t + 1) * 128, c0:c0 + nf])
        xhc = [fp.tile([128, 512], BF16, tag=f"xhc{m}", bufs=2, name=f"xhc{m}") for m in range(4)]
        for m in range(4):
            ps = fps.tile([128, 512], F32, tag="fps", bufs=2, name="ps_xz")
            for kt in range(2):
                nc.tensor.matmul(ps[:, :nf],
                                 lhsT=g.w_in_t[kt][:, m * 128:(m + 1) * 128],
                                 rhs=xin[kt][:, :nf],
                                 start=(kt == 0), stop=(kt == 1))
            nc.scalar.activation(out=xhc[m][:, :nf], in_=ps[:, :nf], func=AF.Copy)
        for cg in range(2):
            psz = fps.tile([ESH, 512], F32, tag="fps", bufs=2, name="ps_z")
            for kt in range(2):
                nc.tensor.matmul(psz[:, :nf],
                                 lhsT=g.w_in_t[kt][:, E + cg * ESH:E + (cg + 1) * ESH],
                                 rhs=xin[kt][:, :nf],
                                 start=(kt == 0), stop=(kt == 1))
            nc.scalar.activation(out=g.z_sl[cg][:, c0:c0 + nf], in_=psz[:, :nf],
                                 func=AF.Copy)
        ps1 = fps.tile([MID, 512], F32, tag="fps", bufs=2, name="ps_pw1")
        for kt in range(4):
            nc.tensor.matmul(ps1[:, :nf], lhsT=g.w_pw1_t[kt][:],
                             rhs=xhc[kt][:, :nf],
                             start=(kt == 0), stop=(kt == 3))
        nc.scalar.activation(out=g.h12[:, c0:c0 + nf], in_=ps1[:, :nf],
                             func=AF.Identity, bias=g.pw1b_t[:])


def _front_conv(g):
    """Depthwise 3x3 on h12 -> accB, in 4 row-bands so each band's ops
    start as soon as the h12 chunks covering its rows (+1 halo) land."""
    nc, fp = g.nc, g.fp
    acc = fp.tile([MID, L], BF16, tag="dwacc")
    acc3 = acc[:].rearrange("p (h w) -> p h w", w=W)
    h3 = g.h12[:].rearrange("p (h w) -> p h w", w=W)
    BAND = 12
    for b0 in range(0, H, BAND):
        b1 = b0 + BAND
        nc.vector.tensor_scalar(out=acc3[:, b0:b1, :],
                                in0=h3[:, b0:b1, :],
                                scalar1=g.dwtap_t[:, 4:5], scalar2=None,
                                op0=OP.mult)
        for ky in range(3):
            for kx in range(3):
                if ky == 1 and kx == 1:
                    continue
                dy, dx = ky - 1, kx - 1
                r0 = max(max(0, -dy), b0)
                r1 = min(H - max(0, dy), b1)
                if r0 >= r1:
                    continue
                w0, w1 = max(0, -dx), W - max(0, dx)
                nc.vector.scalar_tensor_tensor(
                    out=acc3[:, r0:r1, w0:w1],
                    in0=h3[:, r0 + dy:r1 + dy, w0 + dx:w1 + dx],
                    scalar=g.dwtap_t[:, ky * 3 + kx:ky * 3 + kx + 1],
                    in1=acc3[:, r0:r1, w0:w1],
                    op0=OP.mult, op1=OP.add)
        nc.scalar.activation(out=g.accB[:, b0 * W:b1 * W],
                             in_=acc[:, b0 * W:b1 * W], func=AF.Copy)


def _front_stageB(g):
    """pw2+SiLU -> xc chunks -> (xc_sl slice, x_dbl, delta, B/C to DRAM)."""
    nc, fp, fps = g.nc, g.fp, g.fps
    for (c0, nf) in MM_CHUNKS:
        xcc = [fp.tile([128, 512], BF16, tag=f"xcc{m}", bufs=2, name=f"xcc{m}") for m in range(4)]
        for m in range(4):
            ps2 = fps.tile([128, 512], F32, tag="fps", bufs=2, name="ps_pw2")
            nc.tensor.matmul(ps2[:, :nf],
                             lhsT=g.w_pw2_t[:, m * 128:(m + 1) * 128],
                             rhs=g.accB[:, c0:c0 + nf], start=True, stop=True)
            nc.scalar.activation(out=xcc[m][:, :nf], in_=ps2[:, :nf],
                                 func=AF.Silu)
        for cg in range(2):
            nc.sync.dma_start(out=g.xc_sl[cg][:, c0:c0 + nf],
                              in_=xcc[0][cg * ESH:(cg + 1) * ESH, :nf])
        ps3 = fps.tile([64, 512], F32, tag="fps", bufs=2, name="ps_xdbl")
        for kt in range(4):
            nc.tensor.matmul(ps3[:, :nf], lhsT=g.w_xp_t[kt][:],
                             rhs=xcc[kt][:, :nf],
                             start=(kt == 0), stop=(kt == 3))
        nc.scalar.activation(out=g.xdbl[:, c0:c0 + nf], in_=ps3[:, :nf],
                             func=AF.Copy)
        for cg in range(2):
            ps4 = fps.tile([ESH, 512], F32, tag="fps", bufs=2, name="ps_dt")
            nc.tensor.matmul(ps4[:, :nf],
                             lhsT=g.w_dt_t[:, cg * ESH:(cg + 1) * ESH],
                             rhs=g.xdbl[0:R, c0:c0 + nf], start=True, stop=True)
            nc.scalar.activation(out=g.dtlin[cg][:, c0:c0 + nf], in_=ps4[:, :nf],
                                 func=AF.Copy)
    # softplus(v) = ln(1 + exp(v)); |v| < ~10 here so exp cannot overflow.
    # Batched over full L to avoid ACT table reloads inside the chunk loop.
    for cg in range(2):
        nc.scalar.activation(out=g.delta_rep[cg][0:ESH, :], in_=g.dtlin[cg][:],
                             func=AF.Exp, bias=g.spb_t[cg][:])
        nc.scalar.activation(out=g.delta_rep[cg][0:ESH, :],
                             in_=g.delta_rep[cg][0:ESH, :], func=AF.Ln, bias=1.0)
        # duplicate delta rows [0:64] -> [64:128]
        nc.sync.dma_start(out=g.delta_rep[cg][ESH:2 * ESH, :],
                          in_=g.delta_rep[cg][0:ESH, :])
    # B/C rows to DRAM for later partition-broadcast loads
    nc.sync.dma_start(out=g.bsrc[:], in_=g.xdbl[R:R + N, :])
    nc.sync.dma_start(out=g.csrc[:], in_=g.xdbl[R + N:R + 2 * N, :])


def _scan_prep(g, cg):
    """y_acc init (D*u skip), dA exps, per-direction du tiles."""
    nc, sp, wp = g.nc, g.sp, g.wp
    nc.scalar.activation(out=g.y_acc[cg][:], in_=g.xc_sl[cg][:],
                         func=AF.Identity, bias=g.dpb_t[cg][:],
                         scale=g.dp4_t[cg][:])
    for j in range(NJ):
        g.dA[j] = sp.tile([128, L], BF16, tag=f"dA{j}", name=f"dA{j}")
        nc.scalar.activation(out=g.dA[j][:], in_=g.delta_rep[cg][:],
                             func=AF.Exp, scale=g.ascale_t[cg][:, j:j + 1])



def _scan_k(g, cg, k):
    """One direction's du + 8 scan tiles + y accumulation."""
    nc, wp, yps = g.nc, g.wp, g.yps
    xc3 = g.xc_sl[cg][:].rearrange("p (h w) -> p h w", w=W)
    xcT = g.xc_sl[cg][:].rearrange("p (h w) -> p w h", w=W)
    usrc = [xc3, xc3[:, ::-1, ::-1], xcT, xcT[:, ::-1, ::-1]][k]
    u_tmp = wp.tile([ESH, L], BF16, tag="u_tmp", bufs=1)
    u3 = u_tmp[:].rearrange("p (a c) -> p a c", c=W)
    nc.scalar.activation(out=u3, in_=usrc, func=AF.Identity,
                         bias=g.dire_t[cg][:, k:k + 1])
    du = wp.tile([128, L], BF16, tag="du", bufs=2)
    nc.vector.tensor_tensor(out=du[0:ESH, :], in0=g.delta_rep[cg][0:ESH, :],
                            in1=u_tmp[:], op=OP.mult)
    nc.sync.dma_start(out=du[ESH:2 * ESH, :], in_=du[0:ESH, :])
    ypsum = [yps.tile([ESH, HALF], F32, tag=f"yps{h}", name=f"yps{h}") for h in range(2)]
    for j in range(NJ):
        B_t = wp.tile([128, L], BF16, tag="B_t", bufs=2)
        C_t = wp.tile([128, L], BF16, tag="C_t", bufs=2)
        for ns in range(2):
            nc.sync.dma_start(
                out=B_t[ns * ESH:(ns + 1) * ESH, :],
                in_=g.bsrc[2 * j + ns:2 * j + ns + 1, :]
                .to_broadcast((ESH, L)))
            nc.sync.dma_start(
                out=C_t[ns * ESH:(ns + 1) * ESH, :],
                in_=g.csrc[2 * j + ns:2 * j + ns + 1, :]
                .to_broadcast((ESH, L)))
        dbu = wp.tile([128, L], BF16, tag="workA", bufs=2)
        nc.vector.tensor_tensor(out=dbu[:], in0=du[:], in1=B_t[:],
                                op=OP.mult)
        h_t = wp.tile([128, L], BF16, tag="workH", bufs=1)
        nc.vector.tensor_tensor_scan(out=h_t[:], data0=g.dA[j][:],
                                     data1=dbu[:], initial=0.0,
                                     op0=OP.mult, op1=OP.add)
        hc = wp.tile([128, L], BF16, tag="workA", bufs=2)
        nc.vector.tensor_tensor(out=hc[:], in0=h_t[:], in1=C_t[:], op=OP.mult)
        for hh in range(2):
            for (c0, nf) in MM_CHUNKS_HALF:
                nc.tensor.matmul(
                    ypsum[hh][:, c0:c0 + nf],
                    lhsT=g.sel_t[:],
                    rhs=hc[:, hh * HALF + c0:hh * HALF + c0 + nf],
                    start=(j == 0), stop=(j == NJ - 1))
    # accumulate un-permuted ys_k into y_acc
    for hh in range(2):
        pv = ypsum[hh][:]
        if k == 0:
            dst = g.y_acc[cg][:, hh * HALF:(hh + 1) * HALF]
            srcv = pv
        elif k == 1:
            dst = g.y_acc[cg][:, (1 - hh) * HALF:(2 - hh) * HALF]
            srcv = pv[:, ::-1]
        elif k == 2:
            # ys[i], i=a*48+b_ -> l = b_*48+a ; half hh: a in [24hh,24hh+24)
            dst = g.y_acc[cg][:].rearrange("p (bb a) -> p bb a", a=W)[
                :, :, 24 * hh:24 * hh + 24]
            srcv = pv.rearrange("p (a bb) -> p bb a", bb=W)
        else:
            dst = g.y_acc[cg][:].rearrange("p (bb a) -> p bb a", a=W)[
                :, :, 24 * (1 - hh):24 * (1 - hh) + 24]
            srcv = pv.rearrange("p (a bb) -> p bb a", bb=W)[:, ::-1, ::-1]
        nc.vector.tensor_tensor(out=dst, in0=srcv, in1=dst, op=OP.add)


def _finish_yv(g, cg):
    """yv[cg] = y_acc * silu(z) for this channel group."""
    nc, wp = g.nc, g.wp
    sz = wp.tile([ESH, L], BF16, tag="u_tmp", bufs=1)
    nc.scalar.activation(out=sz[:], in_=g.z_sl[cg][:], func=AF.Silu)
    g.yv[cg] = g.sp.tile([ESH, L], BF16, tag=f"yv{cg}", name=f"yv{cg}")
    nc.vector.tensor_tensor(out=g.yv[cg][:], in0=g.y_acc[cg][:], in1=sz[:],
                            op=OP.mult)


def _out_proj(g):
    """out_partial = sum_cg W_out[cg]^T @ yv[cg] (PSUM-accumulated)."""
    nc, wp, fps = g.nc, g.wp, g.fps
    for m in range(2):
        for (c0, nf) in MM_CHUNKS:
            po = fps.tile([128, 512], F32, tag="fps", bufs=2, name="ps_out")
            for cg in range(2):
                nc.tensor.matmul(po[:, :nf],
                                 lhsT=g.w_out_t[cg][:, m * 128:(m + 1) * 128],
                                 rhs=g.yv[cg][:, c0:c0 + nf],
                                 start=(cg == 0), stop=(cg == 1))
            osb = wp.tile([128, 512], F32, tag="osb", bufs=2)
            nc.scalar.activation(out=osb[:, :nf], in_=po[:, :nf], func=AF.Copy)
            nc.sync.dma_start(out=g.out_d[m * 128:(m + 1) * 128, c0:c0 + nf],
                              in_=osb[:, :nf])


def _r32r(a):
    """Round fp32 -> fp32r (TF32-like, 10 explicit mantissa bits)."""
    b = np.ascontiguousarray(a, np.float32).view(np.uint32)
    return (((b.astype(np.uint64) + 0x1000) & 0xFFFFE000)
            .astype(np.uint32).view(np.float32))


def _bf16(a):
    return np.ascontiguousarray(np.asarray(a, np.float32)).astype(
        ml_dtypes.bfloat16)


def _host_prep(inputs):
    x = np.asarray(inputs["x"], np.float32)
    W_pos = np.asarray(inputs["W_pos"], np.float32)
    b_pos = np.asarray(inputs["b_pos"], np.float32)
    W_in = np.asarray(inputs["W_in"], np.float32)
    pw1_w = np.asarray(inputs["pw1_w"], np.float32)
    pw1_b = np.asarray(inputs["pw1_b"], np.float32)
    dw_w = np.asarray(inputs["dw_w"], np.float32)
    pw2_w = np.asarray(inputs["pw2_w"], np.float32)
    W_xproj = np.asarray(inputs["W_xproj"], np.float32)
    W_dt = np.asarray(inputs["W_dt"], np.float32)
    b_dt = np.asarray(inputs["b_dt"], np.float32)
    A_log = np.asarray(inputs["A_log"], np.float32)
    Dp = np.asarray(inputs["Dp"], np.float32)
    dir_emb = np.asarray(inputs["dir_emb"], np.float32)
    W_out = np.asarray(inputs["W_out"], np.float32)

    gy, gx = np.meshgrid(np.arange(H, dtype=np.float32),
                         np.arange(W, dtype=np.float32), indexing="ij")
    coords = np.stack([gy, gx], -1) / (H - 1) * 2 - 1
    pos = (coords.reshape(L, 2) @ W_pos + b_pos).astype(np.float32)

    common = {
        "w_pw1": _bf16(pw1_w.reshape(MID, E).T),
        "pw1b": np.ascontiguousarray(pw1_b.reshape(MID, 1)),
        "dwtap": np.ascontiguousarray(dw_w.reshape(MID, 9)),
    }
    w_pw2_base = pw2_w.reshape(E, MID).T  # (MID, E)
    A = -np.exp(A_log)  # (E, N)
    xp = (x + pos[None]).transpose(0, 2, 1)  # (B, Dm, L)

    sel = np.zeros((2 * ESH, ESH), np.float32)
    for p in range(2 * ESH):
        sel[p, p % ESH] = 1.0
    sel = sel.astype(ml_dtypes.bfloat16)

    in_maps = []
    for c in range(NCORES):
        bcr = c // 4               # this core's batch
        e0 = (c % 4) * 2 * ESH     # this core's 128-channel slice
        sl = slice(e0, e0 + 2 * ESH)
        ascale = np.empty((2, 2 * ESH, NJ), np.float32)
        for cg in range(2):
            A_cg = A[e0 + cg * ESH:e0 + (cg + 1) * ESH]  # (64, 16)
            for p in range(2 * ESH):
                for j in range(NJ):
                    ascale[cg, p, j] = A_cg[p % ESH, 2 * j + p // ESH]
        m = dict(common)
        m["xT"] = _bf16(xp[bcr])
        # channel permutation putting this core's slice at rows [0:128]
        perm = np.concatenate([np.arange(e0, e0 + 2 * ESH),
                               np.arange(0, e0),
                               np.arange(e0 + 2 * ESH, E)])
        m["w_pw2"] = _bf16(w_pw2_base[:, perm])
        m["w_xp"] = _bf16(np.concatenate(
            [W_xproj[perm, :], np.zeros((E, 64 - (R + 2 * N)), np.float32)],
            axis=1))
        m["w_in"] = _bf16(
            np.concatenate([W_in[:, :E], W_in[:, E + e0:E + e0 + 2 * ESH]],
                           axis=1))
        m["w_dt"] = _bf16(W_dt[:, sl])
        m["spb"] = np.ascontiguousarray(
            (2.0 * b_dt[sl]).reshape(2, ESH, 1))
        m["ascale"] = ascale
        m["dire"] = np.ascontiguousarray(
            dir_emb[:, sl].T.reshape(2, ESH, 4))
        m["dp4"] = np.ascontiguousarray((4.0 * Dp[sl]).reshape(2, ESH, 1))
        m["dpb"] = np.ascontiguousarray(
            (Dp[sl] * dir_emb[:, sl].sum(0)).reshape(2, ESH, 1))
        m["w_out"] = _bf16(W_out[sl, :].reshape(2, ESH, DM))
        m["sel"] = sel
        in_maps.append(m)
    return in_maps


_PROGRAM = None
_LAST_RESULTS = None
_LAST_INSTS = None


def _get_program():
    global _PROGRAM
    if _PROGRAM is None:
        _PROGRAM = build_program()
    return _PROGRAM


def kernel(**inputs):
    global _LAST_EXEC_NS, _LAST_RESULTS
    assert int(inputs["H"]) == H and int(inputs["W"]) == W
    in_maps = _host_prep(inputs)
    if TRACE:
        _install_profile_shim()
    res = run_bass_kernel_spmd(_get_program(), in_maps,
                               list(range(NCORES)), trace=TRACE)
    _LAST_EXEC_NS = res.exec_time_ns
    _LAST_RESULTS = res.results
    global _LAST_INSTS
    _LAST_INSTS = res.instructions_and_trace
    out = np.zeros((B, DM, L), np.float32)
    for c, r in enumerate(res.results):
        out[c // 4] += np.asarray(r["out"], np.float32)
    return np.ascontiguousarray(out.transpose(0, 2, 1))
